# revision 1
# baseline (speedup 1.0000x reference)
"""BitNet-style row-parallel linear on 8 TRN2 NeuronCores.

Reference computes: out[b,s,o] = sum_d x[b,s,d] * sign(w[o,d]) + bias[o]
  x: [4, 2048, 4096] f32, w: [4096, 4096] f32, bias: [4096] f32.

Strategy: data-parallel over the 8192 (b*s) rows — each of the 8 cores
computes a 1024-row slice of the output against the full binarized
weight. No collective needed; shards concatenate to the full output.
(The row-parallel/all-reduce hint costs a 128MB all-reduce per core;
sharding M instead makes the partial outputs disjoint.)

TensorE consumes both operands K-major, so the host preps:
  kxm = x_shard.T           [K=4096, M=1024]  (per core)
  kxn = sign(w).T           [K=4096, N=4096]  (same on every core)
The matmul runs in bf16 (weights are exactly +-1 in bf16; x rounds
to ~1e-3 relative) at 1 PE cycle/row — true fp32 is 4x slower, and
float32r (fp22) costs ~9% more wall time in DMA; see DTYPE below.
"""

import numpy as np

B, S, D_IN, D_OUT = 4, 2048, 4096, 4096
NCORES = 8
M_TOTAL = B * S
M_CORE = M_TOTAL // NCORES

import os

_cache = {}

# "f32r" (fp22 multiply, highest precision) or "bf16" (half the DMA
# traffic + fast weight load; weights are exactly representable).
DTYPE = os.environ.get("BK_DTYPE", "bf16")


IMPL = os.environ.get("BK_IMPL", "lib")


def _custom_body(nc, tc, kxm, kxn, out, mm_dt, mybir):
    """x^T stays SBUF-resident; sign(w)^T streams through once.

    Per n-block of 512 output columns, accumulate k into PSUM banks.
    Block 0 sweeps all 8 banks per k-tile (x still streaming in);
    later blocks run one bank at a time so evictions pipeline and the
    tail after the last matmul is a single evict+store.
    """
    P = 128
    KT = D_IN // P          # 32 k tiles
    MT = M_CORE // P        # 8 m tiles
    NW = 512
    NB = D_OUT // NW        # 8 n blocks
    f32 = mybir.dt.float32

    from contextlib import ExitStack
    with ExitStack() as ctx:
        kxm_pool = ctx.enter_context(tc.tile_pool(name="kxm", bufs=1))
        kxn_pool = ctx.enter_context(tc.tile_pool(name="kxn", bufs=9))
        psum_pool = ctx.enter_context(
            tc.tile_pool(name="psum", bufs=8, space="PSUM"))
        out_pool = ctx.enter_context(tc.tile_pool(name="outp", bufs=8))

        def issue_chunk(nb, c, k0, sz):
            # one kxn chunk: k tiles [k0, k0+sz) of n block nb
            t = kxn_pool.tile([P, sz, NW], mm_dt, tag="kxn",
                              name=f"kxn_{nb}_{c}", bufs=24)
            src = kxn[k0 * P:(k0 + sz) * P, nb * NW:(nb + 1) * NW]
            nc.sync.dma_start(
                out=t, in_=src.rearrange("(ko ki) n -> ki ko n", ki=P))
            return [t[:, i, :] for i in range(sz)]

        def issue_chunks(nb, sizes):
            rhs, k0 = [], 0
            for c, sz in enumerate(sizes):
                rhs += issue_chunk(nb, c, k0, sz)
                k0 += sz
            return rhs

        kxm_tiles = {}

        def issue_kxm(k, h):
            kt = kxm_pool.tile([P, M_CORE // 2], mm_dt, tag="kxm",
                               name=f"kxm_{k}_{h}", bufs=2 * KT)
            eng = nc.scalar if h == 0 else nc.gpsimd
            eng.dma_start(out=kt[:, :],
                          in_=kxm[k * P:(k + 1) * P,
                                  h * (M_CORE // 2):(h + 1) * (M_CORE // 2)])
            kxm_tiles[(k, h)] = kt

        def lhsT(k, m):
            h, off = divmod(m, MT // 2)
            return kxm_tiles[(k, h)][:, off * P:(off + 1) * P]

        # Prologue interleave: x low-halves arrive at sweep-A pace on
        # the scalar queue, weight chunks on sync; x high-halves (for
        # sweep B) trail on the gpsimd queue.
        sizes0 = [2, 2, 2, 2, 4, 4, 4, 4, 4, 4]
        rhs0, k0 = [], 0
        issue_kxm(0, 0)
        issue_kxm(1, 0)
        for c, sz in enumerate(sizes0):
            rhs0 += issue_chunk(0, c, k0, sz)
            k0 += sz
            for k in range(min(k0 + 2, KT)):
                if (k, 0) not in kxm_tiles:
                    issue_kxm(k, 0)
            for k in range(min(k0 - 8, KT)):
                if (k, 1) not in kxm_tiles:
                    issue_kxm(k, 1)
        for k in range(KT):
            if (k, 0) not in kxm_tiles:
                issue_kxm(k, 0)
        for k in range(KT):
            if (k, 1) not in kxm_tiles:
                issue_kxm(k, 1)

        next_rhs = rhs0
        for nb in range(NB):
            ncols = slice(nb * NW, (nb + 1) * NW)
            rhs_k = next_rhs
            psums = [psum_pool.tile([P, NW], f32, tag="ps", name=f"ps_{nb}_{i}")
                     for i in range(MT)]
            # Block 0: two 4-bank sweeps matched to the x-half arrival
            # rate; later blocks: one bank at a time (x resident).
            groups = [range(MT // 2), range(MT // 2, MT)] if nb == 0 \
                else [[m] for m in range(MT)]
            for gi, ms in enumerate(groups):
                for k in range(KT):
                    for m in ms:
                        nc.tensor.matmul(
                            psums[m][:, :],
                            lhsT=lhsT(k, m),
                            rhs=rhs_k[k],
                            start=(k == 0), stop=(k == KT - 1))
                if gi == 0 and nb + 1 < NB:
                    next_rhs = issue_chunks(nb + 1, [4] * 8)
                for m in ms:
                    ot = out_pool.tile([P, NW], f32, tag="ot", name=f"ot_{nb}_{m}")
                    nc.vector.tensor_copy(out=ot[:, :], in_=psums[m][:, :])
                    nc.gpsimd.dma_start(
                        out=out[m * P:(m + 1) * P, ncols], in_=ot[:, :])


def _build():
    """Build + compile the 8-core SPMD Bass program once per process."""
    if "nc" in _cache:
        return _cache["nc"]

    import concourse.bacc as bacc
    import concourse.tile as tile
    import concourse.mybir as mybir
    from concourse.kernels.tile_matmul import matmul_tile_kernel

    mm_dt = {"f32r": mybir.dt.float32r, "bf16": mybir.dt.bfloat16}[DTYPE]

    nc = bacc.Bacc("TRN2", target_bir_lowering=False, debug=False,
                   enable_asserts=bool(os.environ.get("BK_ASSERTS")), num_devices=NCORES)
    kxm = nc.dram_tensor("kxm", [D_IN, M_CORE], mm_dt,
                         kind="ExternalInput").ap()
    kxn = nc.dram_tensor("kxn", [D_IN, D_OUT], mm_dt,
                         kind="ExternalInput").ap()
    out = nc.dram_tensor("out", [M_CORE, D_OUT], mybir.dt.float32,
                         kind="ExternalOutput").ap()
    def _warmup(tc):
        # The PE clock is HAM-throttled to 1.2GHz until ~3.4us of
        # sustained matmul activity. The first real matmul can't start
        # until its DMAs land (~13us in), so burn that window warming
        # the clock gate with matmuls on memset tiles; their PSUM bank
        # frees on pool exit before the real kernel allocates.
        from contextlib import ExitStack
        with ExitStack() as ctx:
            wp = ctx.enter_context(tc.tile_pool(name="warm", bufs=1))
            wpp = ctx.enter_context(
                tc.tile_pool(name="warmp", bufs=1, space="PSUM"))
            wdt = mybir.dt.bfloat16
            a = wp.tile([128, 128], wdt)
            b = wp.tile([128, 512], wdt)
            nc.any.memset(a[:, :], 0.0)
            nc.any.memset(b[:, :], 0.0)
            ps = wpp.tile([128, 512], mybir.dt.float32)
            for _ in range(int(os.environ.get("BK_WARM", "12"))):
                nc.tensor.matmul(ps[:, :], lhsT=a[:, :], rhs=b[:, :],
                                 start=True, stop=True)

    if IMPL == "custom":
        with tile.TileContext(nc) as tc:
            _warmup(tc)
            _custom_body(nc, tc, kxm, kxn, out, mm_dt, mybir)
    else:
        kw = {}
        if os.environ.get("BK_MAX_K_TILE"):
            kw["MAX_K_TILE_SIZE"] = int(os.environ["BK_MAX_K_TILE"])
        if os.environ.get("BK_SKIP_K_SNAKE"):
            kw["skip_k_snake"] = True
        if os.environ.get("BK_NO_CACHE_TILES"):
            kw["cache_tiles"] = False
        with tile.TileContext(nc) as tc:
            _warmup(tc)
            matmul_tile_kernel(tc, kxm, kxn, out, **kw)
    nc.compile()
    _cache["nc"] = nc
    return nc


def _prep_inputs(x, weight):
    if DTYPE == "bf16":
        import ml_dtypes
        np_dt = ml_dtypes.bfloat16
    else:
        np_dt = np.float32
    x2d = np.asarray(x, dtype=np.float32).reshape(M_TOTAL, D_IN)
    kxn = np.ascontiguousarray(np.sign(weight, dtype=np.float32).T.astype(np_dt))
    in_maps = []
    for c in range(NCORES):
        kxm = np.ascontiguousarray(x2d[c * M_CORE:(c + 1) * M_CORE].T.astype(np_dt))
        in_maps.append({"kxm": kxm, "kxn": kxn})
    return in_maps


def _run(x, weight, bias, trace=False):
    from concourse.bass_utils import run_bass_kernel_spmd

    nc = _build()
    in_maps = _prep_inputs(x, weight)
    res = run_bass_kernel_spmd(nc, in_maps, core_ids=list(range(NCORES)),
                               trace=trace)
    out = np.concatenate([res.results[c]["out"] for c in range(NCORES)],
                         axis=0)
    bias = np.asarray(bias, dtype=np.float32)
    if np.any(bias):
        out += bias
    return out.reshape(B, S, D_OUT), res


def kernel(x, weight, bias):
    out, _ = _run(x, weight, bias, trace=False)
    return out



# revision 8
# speedup vs baseline: 1.3963x; 1.3963x over previous
"""BitNet-style row-parallel linear on 8 TRN2 NeuronCores.

Reference computes: out[b,s,o] = sum_d x[b,s,d] * sign(w[o,d]) + bias[o]
  x: [4, 2048, 4096] f32, w: [4096, 4096] f32, bias: [4096] f32.

Strategy: data-parallel over the 8192 (b*s) rows — each of the 8 cores
computes a 1024-row slice of the output against the full binarized
weight. No collective needed; shards concatenate to the full output.
(The row-parallel/all-reduce hint costs a 128MB all-reduce per core;
sharding M instead makes the partial outputs disjoint.)

Dtype: fp8 e4m3 with DoubleRow perf mode (2 fp8 weights per PE cell,
2 MACs/cycle -> ~1.8x the bf16 ALU rate). Weights sign(w) are exactly
representable in e4m3; x quantization alone gives rel err 0.0212
(> 2e-2 gate), so the first H_CORR of the 16 double-row k-groups also
accumulate an e4m3 residual r = e4m3(x - e4m3(x)) against the same
weight tiles (residual magnitude <= |x|/16 fits e4m3 directly, no
rescale needed). H_CORR=4 gives exact rel err 0.0185 on the reference
inputs; bias from the reference is zero but still applied on host.

Everything (x^T 4.2MB, sign(w)^T 16.8MB, residual 1MB in e4m3) is
SBUF-resident per core, so after the initial DMA the 1280 matmuls run
back-to-back with no HBM traffic except output eviction.
"""

import os
import numpy as np

B, S, D_IN, D_OUT = 4, 2048, 4096, 4096
NCORES = 8
M_TOTAL = B * S
M_CORE = M_TOTAL // NCORES

_cache = {}

# "fp8e4" (DoubleRow + residual correction), "bf16", or "f32r".
DTYPE = os.environ.get("BK_DTYPE", "fp8e4")
IMPL = os.environ.get("BK_IMPL", "custom")
# Number of double-row k-groups (256 k each) that get the residual
# correction pass. 4 -> rel err 0.0185, 6 -> 0.0159, 0 -> 0.0212.
H_CORR = int(os.environ.get("BK_HCORR", "4"))


def _dr_body(nc, tc, kxm, kxr, kxn, out, mybir):
    """fp8 DoubleRow matmul with residual-corrected accumulation.

    All operands SBUF-resident. Per m-tile: accumulate 16 main +
    H_CORR residual double-row matmuls into 8 PSUM banks (one per
    512-col n-block), then evict via DVE copy + DMA.
    """
    P = 128
    KD = D_IN // 256          # 16 double-row k groups
    MT = M_CORE // P          # 8 m tiles
    NW = 512
    NB = D_OUT // NW          # 8 n blocks
    f32 = mybir.dt.float32
    fp8 = mybir.dt.float8e4
    DR = mybir.MatmulPerfMode.DoubleRow

    from contextlib import ExitStack
    with ExitStack() as ctx:
        w_pool = ctx.enter_context(tc.tile_pool(name="w", bufs=1))
        q_pool = ctx.enter_context(tc.tile_pool(name="q", bufs=1))
        r_pool = ctx.enter_context(tc.tile_pool(name="r", bufs=1))
        psum_pool = ctx.enter_context(
            tc.tile_pool(name="psum", bufs=8, space="PSUM"))
        out_pool = ctx.enter_context(tc.tile_pool(name="outp", bufs=8))

        # Issue all input DMAs up front. q (x^T) and r are small and go
        # on the scalar queue; the 16 x 1MB weight tiles stream on the
        # sync queue and pace m=0's k-loop.
        q_tiles, w_tiles, r_tiles = [], [], []
        for kd in range(KD):
            qt = q_pool.tile([P, 2, M_CORE], fp8, tag="q", name=f"q_{kd}",
                             bufs=KD)
            nc.scalar.dma_start(
                out=qt, in_=kxm[kd * 256:(kd + 1) * 256, :].rearrange(
                    "(ko ki) m -> ki ko m", ki=P))
            q_tiles.append(qt)
            wt = w_pool.tile([P, 2, D_OUT], fp8, tag="w", name=f"w_{kd}",
                             bufs=KD)
            nc.sync.dma_start(
                out=wt, in_=kxn[kd * 256:(kd + 1) * 256, :].rearrange(
                    "(ko ki) n -> ki ko n", ki=P))
            w_tiles.append(wt)
        for kc in range(H_CORR):
            rt = r_pool.tile([P, 2, M_CORE], fp8, tag="r", name=f"r_{kc}",
                             bufs=max(H_CORR, 1))
            nc.scalar.dma_start(
                out=rt, in_=kxr[kc * 256:(kc + 1) * 256, :].rearrange(
                    "(ko ki) m -> ki ko m", ki=P))
            r_tiles.append(rt)

        n_steps = KD + H_CORR
        for m in range(MT):
            ms = slice(m * P, (m + 1) * P)
            psums = [psum_pool.tile([P, NW], f32, tag="ps", name=f"ps_{m}_{nb}")
                     for nb in range(NB)]
            for step in range(n_steps):
                if step < KD:
                    lhs_t, kd = q_tiles[step], step
                else:
                    lhs_t, kd = r_tiles[step - KD], step - KD
                for nb in range(NB):
                    nc.tensor.matmul(
                        psums[nb][:, :],
                        lhsT=lhs_t[:, :, ms],
                        rhs=w_tiles[kd][:, :, nb * NW:(nb + 1) * NW],
                        start=(step == 0), stop=(step == n_steps - 1),
                        perf_mode=DR)
            for nb in range(NB):
                ot = out_pool.tile([P, NW], f32, tag="ot", name=f"ot_{m}_{nb}")
                nc.vector.tensor_copy(out=ot[:, :], in_=psums[nb][:, :])
                nc.gpsimd.dma_start(
                    out=out[ms, nb * NW:(nb + 1) * NW], in_=ot[:, :])


def _bf16_body(nc, tc, kxm, kxn, out, mm_dt, mybir):
    """Previous-best bf16 path (x^T resident, sign(w)^T streamed)."""
    P = 128
    KT = D_IN // P
    MT = M_CORE // P
    NW = 512
    NB = D_OUT // NW
    f32 = mybir.dt.float32

    from contextlib import ExitStack
    with ExitStack() as ctx:
        kxm_pool = ctx.enter_context(tc.tile_pool(name="kxm", bufs=1))
        kxn_pool = ctx.enter_context(tc.tile_pool(name="kxn", bufs=9))
        psum_pool = ctx.enter_context(
            tc.tile_pool(name="psum", bufs=8, space="PSUM"))
        out_pool = ctx.enter_context(tc.tile_pool(name="outp", bufs=8))

        def issue_chunk(nb, c, k0, sz):
            t = kxn_pool.tile([P, sz, NW], mm_dt, tag="kxn",
                              name=f"kxn_{nb}_{c}", bufs=24)
            src = kxn[k0 * P:(k0 + sz) * P, nb * NW:(nb + 1) * NW]
            nc.sync.dma_start(
                out=t, in_=src.rearrange("(ko ki) n -> ki ko n", ki=P))
            return [t[:, i, :] for i in range(sz)]

        def issue_chunks(nb, sizes):
            rhs, k0 = [], 0
            for c, sz in enumerate(sizes):
                rhs += issue_chunk(nb, c, k0, sz)
                k0 += sz
            return rhs

        kxm_tiles = {}

        def issue_kxm(k, h):
            kt = kxm_pool.tile([P, M_CORE // 2], mm_dt, tag="kxm",
                               name=f"kxm_{k}_{h}", bufs=2 * KT)
            eng = nc.scalar if h == 0 else nc.gpsimd
            eng.dma_start(out=kt[:, :],
                          in_=kxm[k * P:(k + 1) * P,
                                  h * (M_CORE // 2):(h + 1) * (M_CORE // 2)])
            kxm_tiles[(k, h)] = kt

        def lhsT(k, m):
            h, off = divmod(m, MT // 2)
            return kxm_tiles[(k, h)][:, off * P:(off + 1) * P]

        sizes0 = [2, 2, 2, 2, 4, 4, 4, 4, 4, 4]
        rhs0, k0 = [], 0
        issue_kxm(0, 0)
        issue_kxm(1, 0)
        for c, sz in enumerate(sizes0):
            rhs0 += issue_chunk(0, c, k0, sz)
            k0 += sz
            for k in range(min(k0 + 2, KT)):
                if (k, 0) not in kxm_tiles:
                    issue_kxm(k, 0)
            for k in range(min(k0 - 8, KT)):
                if (k, 1) not in kxm_tiles:
                    issue_kxm(k, 1)
        for k in range(KT):
            if (k, 0) not in kxm_tiles:
                issue_kxm(k, 0)
        for k in range(KT):
            if (k, 1) not in kxm_tiles:
                issue_kxm(k, 1)

        next_rhs = rhs0
        for nb in range(NB):
            ncols = slice(nb * NW, (nb + 1) * NW)
            rhs_k = next_rhs
            psums = [psum_pool.tile([P, NW], f32, tag="ps", name=f"ps_{nb}_{i}")
                     for i in range(MT)]
            groups = [range(MT // 2), range(MT // 2, MT)] if nb == 0 \
                else [[m] for m in range(MT)]
            for gi, ms in enumerate(groups):
                for k in range(KT):
                    for m in ms:
                        nc.tensor.matmul(
                            psums[m][:, :],
                            lhsT=lhsT(k, m),
                            rhs=rhs_k[k],
                            start=(k == 0), stop=(k == KT - 1))
                if gi == 0 and nb + 1 < NB:
                    next_rhs = issue_chunks(nb + 1, [4] * 8)
                for m in ms:
                    ot = out_pool.tile([P, NW], f32, tag="ot", name=f"ot_{nb}_{m}")
                    nc.vector.tensor_copy(out=ot[:, :], in_=psums[m][:, :])
                    nc.gpsimd.dma_start(
                        out=out[m * P:(m + 1) * P, ncols], in_=ot[:, :])


def _build():
    """Build + compile the 8-core SPMD Bass program once per process."""
    if "nc" in _cache:
        return _cache["nc"]

    import concourse.bacc as bacc
    import concourse.tile as tile
    import concourse.mybir as mybir
    from concourse.kernels.tile_matmul import matmul_tile_kernel

    mm_dt = {"f32r": mybir.dt.float32r, "bf16": mybir.dt.bfloat16,
             "fp8e4": mybir.dt.float8e4}[DTYPE]

    nc = bacc.Bacc("TRN2", target_bir_lowering=False, debug=False,
                   enable_asserts=bool(os.environ.get("BK_ASSERTS")),
                   num_devices=NCORES)
    kxm = nc.dram_tensor("kxm", [D_IN, M_CORE], mm_dt,
                         kind="ExternalInput").ap()
    kxn = nc.dram_tensor("kxn", [D_IN, D_OUT], mm_dt,
                         kind="ExternalInput").ap()
    kxr = None
    if DTYPE == "fp8e4" and IMPL == "custom":
        kxr = nc.dram_tensor("kxr", [max(H_CORR, 1) * 256, M_CORE], mm_dt,
                             kind="ExternalInput").ap()
    out = nc.dram_tensor("out", [M_CORE, D_OUT], mybir.dt.float32,
                         kind="ExternalOutput").ap()

    def _warmup(tc):
        # The PE clock is HAM-throttled to 1.2GHz until ~3.4us of
        # sustained matmul activity. Burn the initial DMA window
        # warming the clock gate; the PSUM bank frees on pool exit.
        from contextlib import ExitStack
        with ExitStack() as ctx:
            wp = ctx.enter_context(tc.tile_pool(name="warm", bufs=1))
            wpp = ctx.enter_context(
                tc.tile_pool(name="warmp", bufs=1, space="PSUM"))
            wdt = mybir.dt.bfloat16
            a = wp.tile([128, 128], wdt)
            b = wp.tile([128, 512], wdt)
            nc.any.memset(a[:, :], 0.0)
            nc.any.memset(b[:, :], 0.0)
            ps = wpp.tile([128, 512], mybir.dt.float32)
            for _ in range(int(os.environ.get("BK_WARM", "12"))):
                nc.tensor.matmul(ps[:, :], lhsT=a[:, :], rhs=b[:, :],
                                 start=True, stop=True)

    if IMPL == "custom" and DTYPE == "fp8e4":
        with tile.TileContext(nc) as tc:
            _warmup(tc)
            _dr_body(nc, tc, kxm, kxr, kxn, out, mybir)
    elif IMPL == "custom":
        with tile.TileContext(nc) as tc:
            _warmup(tc)
            _bf16_body(nc, tc, kxm, kxn, out, mm_dt, mybir)
    else:
        kw = {}
        if os.environ.get("BK_MAX_K_TILE"):
            kw["MAX_K_TILE_SIZE"] = int(os.environ["BK_MAX_K_TILE"])
        if os.environ.get("BK_SKIP_K_SNAKE"):
            kw["skip_k_snake"] = True
        if os.environ.get("BK_NO_CACHE_TILES"):
            kw["cache_tiles"] = False
        with tile.TileContext(nc) as tc:
            _warmup(tc)
            matmul_tile_kernel(tc, kxm, kxn, out, **kw)
    nc.compile()
    _cache["nc"] = nc
    return nc


def _prep_inputs(x, weight):
    import ml_dtypes
    if DTYPE == "bf16":
        np_dt = ml_dtypes.bfloat16
    elif DTYPE == "fp8e4":
        np_dt = ml_dtypes.float8_e4m3
    else:
        np_dt = np.float32
    x2d = np.asarray(x, dtype=np.float32).reshape(M_TOTAL, D_IN)
    kxn = np.ascontiguousarray(
        np.sign(weight, dtype=np.float32).T.astype(np_dt))
    in_maps = []
    for c in range(NCORES):
        xs = x2d[c * M_CORE:(c + 1) * M_CORE].T  # [D_IN, M_CORE]
        kxm = np.ascontiguousarray(xs.astype(np_dt))
        im = {"kxm": kxm, "kxn": kxn}
        if DTYPE == "fp8e4" and IMPL == "custom":
            kc = max(H_CORR, 1) * 256
            resid = xs[:kc] - kxm[:kc].astype(np.float32)
            im["kxr"] = np.ascontiguousarray(resid.astype(np_dt))
            if H_CORR == 0:
                im["kxr"][:] = 0
        in_maps.append(im)
    return in_maps


def _run(x, weight, bias, trace=False):
    from concourse.bass_utils import run_bass_kernel_spmd

    nc = _build()
    in_maps = _prep_inputs(x, weight)
    res = run_bass_kernel_spmd(nc, in_maps, core_ids=list(range(NCORES)),
                               trace=trace)
    out = np.concatenate([res.results[c]["out"] for c in range(NCORES)],
                         axis=0)
    bias = np.asarray(bias, dtype=np.float32)
    if np.any(bias):
        out += bias
    return out.reshape(B, S, D_OUT), res


def kernel(x, weight, bias):
    out, _ = _run(x, weight, bias, trace=False)
    return out


# revision 10
# speedup vs baseline: 1.4885x; 1.0660x over previous
"""BitNet-style row-parallel linear on 8 TRN2 NeuronCores.

Reference computes: out[b,s,o] = sum_d x[b,s,d] * sign(w[o,d]) + bias[o]
  x: [4, 2048, 4096] f32, w: [4096, 4096] f32, bias: [4096] f32.

Strategy: data-parallel over the 8192 (b*s) rows — each of the 8 cores
computes a 1024-row slice of the output against the full binarized
weight. No collective needed; shards concatenate to the full output.
(The row-parallel/all-reduce hint costs a 128MB all-reduce per core;
sharding M instead makes the partial outputs disjoint.)

Dtype: fp8 e4m3 with DoubleRow perf mode (2 fp8 weights per PE cell,
2 MACs/cycle -> ~1.8x the bf16 ALU rate). Weights sign(w) are exactly
representable in e4m3; x quantization alone gives rel err 0.0212
(> 2e-2 gate), so the first H_CORR of the 16 double-row k-groups also
accumulate an e4m3 residual r = e4m3(x - e4m3(x)) against the same
weight tiles (residual magnitude <= |x|/16 fits e4m3 directly, no
rescale needed). H_CORR=4 gives exact rel err 0.0185 on the reference
inputs; bias from the reference is zero but still applied on host.

Everything (x^T 4.2MB, sign(w)^T 16.8MB, residual 1MB in e4m3) is
SBUF-resident per core, so after the initial DMA the 1280 matmuls run
back-to-back with no HBM traffic except output eviction.
"""

import os
import numpy as np

B, S, D_IN, D_OUT = 4, 2048, 4096, 4096
NCORES = 8
M_TOTAL = B * S
M_CORE = M_TOTAL // NCORES

_cache = {}

# "fp8e4" (DoubleRow + residual correction), "bf16", or "f32r".
DTYPE = os.environ.get("BK_DTYPE", "fp8e4")
IMPL = os.environ.get("BK_IMPL", "custom")
# Number of double-row k-groups (256 k each) that get the residual
# correction pass. 4 -> rel err 0.0185, 6 -> 0.0159, 0 -> 0.0212.
H_CORR = int(os.environ.get("BK_HCORR", "4"))


def _dr_body(nc, tc, kxm, kxr, kxn, out, mybir):
    """fp8 DoubleRow matmul with residual-corrected accumulation.

    All operands SBUF-resident, loaded in a DMA order matched to the
    compute schedule so the PE never waits on HBM:
      - W (sign(w)^T) splits into 32 tiles (16 double-k groups x 2
        n-halves, 0.5MB each): n-half 0 streams first.
      - q (x^T) and r (residual) split by m-pair (64KB tiles) so the
        first chain group's per-k DMA need (0.56MB) arrives faster
        than its 8 matmuls (1.73us) execute.
    Chain groups: 8 PSUM banks hold the accumulation chains of one
    (m-pair x n-half) group = 2m x 4nb; groups run in an order whose
    W/q needs track the DMA stream.  Each chain is 16 main + H_CORR
    residual DoubleRow matmuls; corrections interleave right after
    their main k-group (their W tile is then already resident).
    """
    P = 128
    KD = D_IN // 256          # 16 double-row k groups
    MT = M_CORE // P          # 8 m tiles
    NW = 512
    NB = D_OUT // NW          # 8 n blocks
    NHALF = 2048
    MP = 256                  # m-pair width (2 m tiles)
    f32 = mybir.dt.float32
    fp8 = mybir.dt.float8e4
    DR = mybir.MatmulPerfMode.DoubleRow

    from contextlib import ExitStack
    with ExitStack() as ctx:
        w_pool = ctx.enter_context(tc.tile_pool(name="w", bufs=1))
        q_pool = ctx.enter_context(tc.tile_pool(name="q", bufs=1))
        r_pool = ctx.enter_context(tc.tile_pool(name="r", bufs=1))
        psum_pool = ctx.enter_context(
            tc.tile_pool(name="psum", bufs=8, space="PSUM"))
        out_pool = ctx.enter_context(tc.tile_pool(name="outp", bufs=8))

        w_tiles = {}   # (kd, h) -> [P, 2, NHALF]
        q_tiles = {}   # (kd, mp) -> [P, 2, MP]
        r_tiles = {}   # (kc, mp) -> [P, 2, MP]

        def load_w(kd, h):
            wt = w_pool.tile([P, 2, NHALF], fp8, tag="w",
                             name=f"w_{kd}_{h}", bufs=2 * KD)
            src = kxn[kd * 256:(kd + 1) * 256,
                      h * NHALF:(h + 1) * NHALF]
            nc.sync.dma_start(
                out=wt, in_=src.rearrange("(ko ki) n -> ki ko n", ki=P))
            w_tiles[(kd, h)] = wt

        def load_q(kd, mp):
            qt = q_pool.tile([P, 2, MP], fp8, tag="q",
                             name=f"q_{kd}_{mp}", bufs=4 * KD)
            src = kxm[kd * 256:(kd + 1) * 256, mp * MP:(mp + 1) * MP]
            nc.scalar.dma_start(
                out=qt, in_=src.rearrange("(ko ki) m -> ki ko m", ki=P))
            q_tiles[(kd, mp)] = qt

        def load_r(kc, mp):
            rt = r_pool.tile([P, 2, MP], fp8, tag="r",
                             name=f"r_{kc}_{mp}", bufs=4 * max(H_CORR, 1))
            src = kxr[kc * 256:(kc + 1) * 256, mp * MP:(mp + 1) * MP]
            nc.scalar.dma_start(
                out=rt, in_=src.rearrange("(ko ki) m -> ki ko m", ki=P))
            r_tiles[(kc, mp)] = rt

        # DMA issue order.  Sync queue: W n-half 0 (kd 0..15) then
        # n-half 1.  Scalar queue: q/r for m-pair 0 in step order, then
        # m-pairs 1..3.
        for h in range(2):
            for kd in range(KD):
                load_w(kd, h)
        for mp in range(4):
            for kd in range(KD):
                load_q(kd, mp)
                if kd < H_CORR:
                    load_r(kd, mp)

        # Chain steps: main k-group kd, with its residual correction
        # right after for kd < H_CORR.
        steps = []
        for kd in range(KD):
            steps.append((q_tiles, kd))
            if kd < H_CORR:
                steps.append((r_tiles, kd))

        out_q = [nc.gpsimd, nc.scalar, nc.sync]

        def run_group(mp, h, nbs=None):
            """Accumulation chains for (m-pair mp) x (n-blocks of half h)."""
            nbs = range(h * 4, h * 4 + 4) if nbs is None else nbs
            chains = [(m, nb) for m in (2 * mp, 2 * mp + 1) for nb in nbs]
            psums = {c: psum_pool.tile([P, NW], f32, tag="ps",
                                       name=f"ps_{c[0]}_{c[1]}")
                     for c in chains}
            for si, (tiles, kd) in enumerate(steps):
                for m, nb in chains:
                    nc.tensor.matmul(
                        psums[(m, nb)][:, :],
                        lhsT=tiles[(kd, mp)][:, :, (m % 2) * P:(m % 2 + 1) * P],
                        rhs=w_tiles[(kd, h)][:, :,
                                             (nb - h * 4) * NW:
                                             (nb - h * 4 + 1) * NW],
                        start=(si == 0), stop=(si == len(steps) - 1),
                        perf_mode=DR)
            for j, (m, nb) in enumerate(chains):
                ot = out_pool.tile([P, NW], f32, tag="ot",
                                   name=f"ot_{m}_{nb}")
                nc.vector.tensor_copy(out=ot[:, :], in_=psums[(m, nb)][:, :])
                out_q[j % 3].dma_start(
                    out=out[m * P:(m + 1) * P, nb * NW:(nb + 1) * NW],
                    in_=ot[:, :])

        # Group order follows the DMA stream: (mp0,h0) while W-h0 and
        # q-mp0 stream, (mp0,h1) next, then all-resident groups.
        for mp, h in [(0, 0), (0, 1), (1, 0), (1, 1),
                      (2, 0), (2, 1), (3, 0)]:
            run_group(mp, h)
        # Final group in two 4-chain halves so its eviction + output
        # DMA overlap the remaining matmuls.
        run_group(3, 1, nbs=[4, 5])
        run_group(3, 1, nbs=[6, 7])


def _bf16_body(nc, tc, kxm, kxn, out, mm_dt, mybir):
    """Previous-best bf16 path (x^T resident, sign(w)^T streamed)."""
    P = 128
    KT = D_IN // P
    MT = M_CORE // P
    NW = 512
    NB = D_OUT // NW
    f32 = mybir.dt.float32

    from contextlib import ExitStack
    with ExitStack() as ctx:
        kxm_pool = ctx.enter_context(tc.tile_pool(name="kxm", bufs=1))
        kxn_pool = ctx.enter_context(tc.tile_pool(name="kxn", bufs=9))
        psum_pool = ctx.enter_context(
            tc.tile_pool(name="psum", bufs=8, space="PSUM"))
        out_pool = ctx.enter_context(tc.tile_pool(name="outp", bufs=8))

        def issue_chunk(nb, c, k0, sz):
            t = kxn_pool.tile([P, sz, NW], mm_dt, tag="kxn",
                              name=f"kxn_{nb}_{c}", bufs=24)
            src = kxn[k0 * P:(k0 + sz) * P, nb * NW:(nb + 1) * NW]
            nc.sync.dma_start(
                out=t, in_=src.rearrange("(ko ki) n -> ki ko n", ki=P))
            return [t[:, i, :] for i in range(sz)]

        def issue_chunks(nb, sizes):
            rhs, k0 = [], 0
            for c, sz in enumerate(sizes):
                rhs += issue_chunk(nb, c, k0, sz)
                k0 += sz
            return rhs

        kxm_tiles = {}

        def issue_kxm(k, h):
            kt = kxm_pool.tile([P, M_CORE // 2], mm_dt, tag="kxm",
                               name=f"kxm_{k}_{h}", bufs=2 * KT)
            eng = nc.scalar if h == 0 else nc.gpsimd
            eng.dma_start(out=kt[:, :],
                          in_=kxm[k * P:(k + 1) * P,
                                  h * (M_CORE // 2):(h + 1) * (M_CORE // 2)])
            kxm_tiles[(k, h)] = kt

        def lhsT(k, m):
            h, off = divmod(m, MT // 2)
            return kxm_tiles[(k, h)][:, off * P:(off + 1) * P]

        sizes0 = [2, 2, 2, 2, 4, 4, 4, 4, 4, 4]
        rhs0, k0 = [], 0
        issue_kxm(0, 0)
        issue_kxm(1, 0)
        for c, sz in enumerate(sizes0):
            rhs0 += issue_chunk(0, c, k0, sz)
            k0 += sz
            for k in range(min(k0 + 2, KT)):
                if (k, 0) not in kxm_tiles:
                    issue_kxm(k, 0)
            for k in range(min(k0 - 8, KT)):
                if (k, 1) not in kxm_tiles:
                    issue_kxm(k, 1)
        for k in range(KT):
            if (k, 0) not in kxm_tiles:
                issue_kxm(k, 0)
        for k in range(KT):
            if (k, 1) not in kxm_tiles:
                issue_kxm(k, 1)

        next_rhs = rhs0
        for nb in range(NB):
            ncols = slice(nb * NW, (nb + 1) * NW)
            rhs_k = next_rhs
            psums = [psum_pool.tile([P, NW], f32, tag="ps", name=f"ps_{nb}_{i}")
                     for i in range(MT)]
            groups = [range(MT // 2), range(MT // 2, MT)] if nb == 0 \
                else [[m] for m in range(MT)]
            for gi, ms in enumerate(groups):
                for k in range(KT):
                    for m in ms:
                        nc.tensor.matmul(
                            psums[m][:, :],
                            lhsT=lhsT(k, m),
                            rhs=rhs_k[k],
                            start=(k == 0), stop=(k == KT - 1))
                if gi == 0 and nb + 1 < NB:
                    next_rhs = issue_chunks(nb + 1, [4] * 8)
                for m in ms:
                    ot = out_pool.tile([P, NW], f32, tag="ot", name=f"ot_{nb}_{m}")
                    nc.vector.tensor_copy(out=ot[:, :], in_=psums[m][:, :])
                    nc.gpsimd.dma_start(
                        out=out[m * P:(m + 1) * P, ncols], in_=ot[:, :])


def _build():
    """Build + compile the 8-core SPMD Bass program once per process."""
    if "nc" in _cache:
        return _cache["nc"]

    import concourse.bacc as bacc
    import concourse.tile as tile
    import concourse.mybir as mybir
    from concourse.kernels.tile_matmul import matmul_tile_kernel

    mm_dt = {"f32r": mybir.dt.float32r, "bf16": mybir.dt.bfloat16,
             "fp8e4": mybir.dt.float8e4}[DTYPE]

    nc = bacc.Bacc("TRN2", target_bir_lowering=False, debug=False,
                   enable_asserts=bool(os.environ.get("BK_ASSERTS")),
                   num_devices=NCORES)
    kxm = nc.dram_tensor("kxm", [D_IN, M_CORE], mm_dt,
                         kind="ExternalInput").ap()
    kxn = nc.dram_tensor("kxn", [D_IN, D_OUT], mm_dt,
                         kind="ExternalInput").ap()
    kxr = None
    if DTYPE == "fp8e4" and IMPL == "custom":
        kxr = nc.dram_tensor("kxr", [max(H_CORR, 1) * 256, M_CORE], mm_dt,
                             kind="ExternalInput").ap()
    out = nc.dram_tensor("out", [M_CORE, D_OUT], mybir.dt.float32,
                         kind="ExternalOutput").ap()

    def _warmup(tc):
        # The PE clock is HAM-throttled to 1.2GHz until ~3.4us of
        # sustained matmul activity. Burn the initial DMA window
        # warming the clock gate; the PSUM bank frees on pool exit.
        from contextlib import ExitStack
        with ExitStack() as ctx:
            wp = ctx.enter_context(tc.tile_pool(name="warm", bufs=1))
            wpp = ctx.enter_context(
                tc.tile_pool(name="warmp", bufs=1, space="PSUM"))
            wdt = mybir.dt.bfloat16
            a = wp.tile([128, 128], wdt)
            b = wp.tile([128, 512], wdt)
            nc.any.memset(a[:, :], 0.0)
            nc.any.memset(b[:, :], 0.0)
            ps = wpp.tile([128, 512], mybir.dt.float32)
            for _ in range(int(os.environ.get("BK_WARM", "12"))):
                nc.tensor.matmul(ps[:, :], lhsT=a[:, :], rhs=b[:, :],
                                 start=True, stop=True)

    if IMPL == "custom" and DTYPE == "fp8e4":
        with tile.TileContext(nc) as tc:
            _warmup(tc)
            _dr_body(nc, tc, kxm, kxr, kxn, out, mybir)
    elif IMPL == "custom":
        with tile.TileContext(nc) as tc:
            _warmup(tc)
            _bf16_body(nc, tc, kxm, kxn, out, mm_dt, mybir)
    else:
        kw = {}
        if os.environ.get("BK_MAX_K_TILE"):
            kw["MAX_K_TILE_SIZE"] = int(os.environ["BK_MAX_K_TILE"])
        if os.environ.get("BK_SKIP_K_SNAKE"):
            kw["skip_k_snake"] = True
        if os.environ.get("BK_NO_CACHE_TILES"):
            kw["cache_tiles"] = False
        with tile.TileContext(nc) as tc:
            _warmup(tc)
            matmul_tile_kernel(tc, kxm, kxn, out, **kw)
    nc.compile()
    _cache["nc"] = nc
    return nc


def _prep_inputs(x, weight):
    import ml_dtypes
    if DTYPE == "bf16":
        np_dt = ml_dtypes.bfloat16
    elif DTYPE == "fp8e4":
        np_dt = ml_dtypes.float8_e4m3
    else:
        np_dt = np.float32
    x2d = np.asarray(x, dtype=np.float32).reshape(M_TOTAL, D_IN)
    kxn = np.ascontiguousarray(
        np.sign(weight, dtype=np.float32).T.astype(np_dt))
    in_maps = []
    for c in range(NCORES):
        xs = x2d[c * M_CORE:(c + 1) * M_CORE].T  # [D_IN, M_CORE]
        kxm = np.ascontiguousarray(xs.astype(np_dt))
        im = {"kxm": kxm, "kxn": kxn}
        if DTYPE == "fp8e4" and IMPL == "custom":
            kc = max(H_CORR, 1) * 256
            resid = xs[:kc] - kxm[:kc].astype(np.float32)
            im["kxr"] = np.ascontiguousarray(resid.astype(np_dt))
            if H_CORR == 0:
                im["kxr"][:] = 0
        in_maps.append(im)
    return in_maps


def _run(x, weight, bias, trace=False):
    from concourse.bass_utils import run_bass_kernel_spmd

    nc = _build()
    in_maps = _prep_inputs(x, weight)
    res = run_bass_kernel_spmd(nc, in_maps, core_ids=list(range(NCORES)),
                               trace=trace)
    out = np.concatenate([res.results[c]["out"] for c in range(NCORES)],
                         axis=0)
    bias = np.asarray(bias, dtype=np.float32)
    if np.any(bias):
        out += bias
    return out.reshape(B, S, D_OUT), res


def kernel(x, weight, bias):
    out, _ = _run(x, weight, bias, trace=False)
    return out


# revision 17
# speedup vs baseline: 1.5491x; 1.0407x over previous
"""BitNet-style row-parallel linear on 8 TRN2 NeuronCores.

Reference computes: out[b,s,o] = sum_d x[b,s,d] * sign(w[o,d]) + bias[o]
  x: [4, 2048, 4096] f32, w: [4096, 4096] f32, bias: [4096] f32.

Strategy: data-parallel over the 8192 (b*s) rows — each of the 8 cores
computes a 1024-row slice of the output against the full binarized
weight. No collective needed; shards concatenate to the full output.
(The row-parallel/all-reduce hint costs a 128MB all-reduce per core;
sharding M instead makes the partial outputs disjoint.)

Dtype: fp8 e4m3 with DoubleRow perf mode (2 fp8 weights per PE cell,
2 MACs/cycle -> ~1.8x the bf16 ALU rate). Weights sign(w) are exactly
representable in e4m3; x quantization alone gives rel err 0.0212
(> 2e-2 gate), so the first H_CORR of the 16 double-row k-groups also
accumulate an e4m3 residual r = e4m3(x - e4m3(x)) against the same
weight tiles (residual magnitude <= |x|/16 fits e4m3 directly, no
rescale needed). H_CORR=4 gives exact rel err 0.0185 on the reference
inputs; bias from the reference is zero but still applied on host.

Everything (x^T 4.2MB, sign(w)^T 16.8MB, residual 1MB in e4m3) is
SBUF-resident per core, so after the initial DMA the 1280 matmuls run
back-to-back with no HBM traffic except output eviction.
"""

import os
import numpy as np

B, S, D_IN, D_OUT = 4, 2048, 4096, 4096
NCORES = 8
M_TOTAL = B * S
M_CORE = M_TOTAL // NCORES

_cache = {}

# "fp8e4" (DoubleRow + residual correction), "bf16", or "f32r".
DTYPE = os.environ.get("BK_DTYPE", "fp8e4")
IMPL = os.environ.get("BK_IMPL", "custom")
# Number of double-row k-groups (256 k each) that get the residual
# correction pass. 4 -> rel err 0.0185, 6 -> 0.0159, 0 -> 0.0212.
H_CORR = int(os.environ.get("BK_HCORR", "4"))


def _dr_body(nc, tc, kxm, kxr, kxn, out, mybir):
    """fp8 DoubleRow matmul with residual-corrected accumulation.

    All operands SBUF-resident, loaded in a DMA order matched to the
    compute schedule so the PE never waits on HBM:
      - W (sign(w)^T) splits into 32 tiles (16 double-k groups x 2
        n-halves, 0.5MB each): n-half 0 streams first.
      - q (x^T) and r (residual) split by m-pair (64KB tiles) so the
        first chain group's per-k DMA need (0.56MB) arrives faster
        than its 8 matmuls (1.73us) execute.
    Chain groups: 8 PSUM banks hold the accumulation chains of one
    (m-pair x n-half) group = 2m x 4nb; groups run in an order whose
    W/q needs track the DMA stream.  Each chain is 16 main + H_CORR
    residual DoubleRow matmuls; corrections interleave right after
    their main k-group (their W tile is then already resident).
    """
    P = 128
    KD = D_IN // 256          # 16 double-row k groups
    MT = M_CORE // P          # 8 m tiles
    NW = 512
    NB = D_OUT // NW          # 8 n blocks
    NHALF = 2048
    MP = 256                  # m-pair width (2 m tiles)
    f32 = mybir.dt.float32
    fp8 = mybir.dt.float8e4
    DR = mybir.MatmulPerfMode.DoubleRow

    from contextlib import ExitStack
    with ExitStack() as ctx:
        w_pool = ctx.enter_context(tc.tile_pool(name="w", bufs=1))
        q_pool = ctx.enter_context(tc.tile_pool(name="q", bufs=1))
        r_pool = ctx.enter_context(tc.tile_pool(name="r", bufs=1))
        psum_pool = ctx.enter_context(
            tc.tile_pool(name="psum", bufs=8, space="PSUM"))
        out_pool = ctx.enter_context(tc.tile_pool(name="outp", bufs=8))

        w_tiles = {}   # (kd, h) -> [P, 2, NHALF]
        q_tiles = {}   # (kd, mp) -> [P, 2, MP]
        r_tiles = {}   # (kc, mp) -> [P, 2, MP]

        # DRAM is host-pre-tiled so every DMA is a contiguous block
        # with >=512B per-partition lines (W: 4KB) and no gather cost.
        def load_w(kd, h, eng):
            wt = w_pool.tile([P, 2, NHALF], fp8, tag="w",
                             name=f"w_{kd}_{h}", bufs=2 * KD)
            base = (h * KD + kd) * P
            eng.dma_start(
                out=wt, in_=kxn[base:base + P, :].rearrange(
                    "p (ko n) -> p ko n", ko=2))
            w_tiles[(kd, h)] = wt

        def load_q(kd, mp, eng):
            qt = q_pool.tile([P, 2, MP], fp8, tag="q",
                             name=f"q_{kd}_{mp}", bufs=4 * KD)
            base = (mp * KD + kd) * P
            eng.dma_start(
                out=qt, in_=kxm[base:base + P, :].rearrange(
                    "p (ko m) -> p ko m", ko=2))
            q_tiles[(kd, mp)] = qt

        def load_r(kc, mp, eng):
            rt = r_pool.tile([P, 2, MP], fp8, tag="r",
                             name=f"r_{kc}_{mp}", bufs=4 * max(H_CORR, 1))
            base = (mp * max(H_CORR, 1) + kc) * P
            eng.dma_start(
                out=rt, in_=kxr[base:base + P, :].rearrange(
                    "p (ko m) -> p ko m", ko=2))
            r_tiles[(kc, mp)] = rt

        # Sync queue carries a single just-in-time stream matched to
        # group (mp0,h0)'s consumption: per k-group its q (and r)
        # slices land right before its 0.5MB W tile, then W n-half 1
        # follows for group (mp0,h1).  q/r of m-pairs 1..3 go on the
        # gpsimd queue, which is gated below so they don't steal HBM
        # bandwidth from this stream.
        for kd in range(KD):
            load_q(kd, 0, nc.sync)
            if kd < H_CORR:
                load_r(kd, 0, nc.sync)
            load_w(kd, 0, nc.sync)
        for kd in range(KD):
            load_w(kd, 1, nc.sync)

        # Chain steps: main k-group kd, with its residual correction
        # right after for kd < H_CORR.
        steps = []
        for kd in range(KD):
            steps.append((q_tiles, kd))
            if kd < H_CORR:
                steps.append((r_tiles, kd))

        def run_group(mp, h, nbs=None):
            """Accumulation chains for (m-pair mp) x (n-blocks of half h)."""
            nbs = range(h * 4, h * 4 + 4) if nbs is None else nbs
            chains = [(m, nb) for m in (2 * mp, 2 * mp + 1) for nb in nbs]
            psums = {c: psum_pool.tile([P, NW], f32, tag="ps",
                                       name=f"ps_{c[0]}_{c[1]}")
                     for c in chains}
            for si, (tiles, kd) in enumerate(steps):
                for m, nb in chains:
                    nc.tensor.matmul(
                        psums[(m, nb)][:, :],
                        lhsT=tiles[(kd, mp)][:, :, (m % 2) * P:(m % 2 + 1) * P],
                        rhs=w_tiles[(kd, h)][:, :,
                                             (nb - h * 4) * NW:
                                             (nb - h * 4 + 1) * NW],
                        start=(si == 0), stop=(si == len(steps) - 1),
                        perf_mode=DR)
            for j, (m, nb) in enumerate(chains):
                ot = out_pool.tile([P, NW], f32, tag="ot",
                                   name=f"ot_{m}_{nb}")
                nc.vector.tensor_copy(out=ot[:, :], in_=psums[(m, nb)][:, :])
                (nc.scalar if j % 2 else nc.gpsimd).dma_start(
                    out=out[m * P:(m + 1) * P, nb * NW:(nb + 1) * NW],
                    in_=ot[:, :])

        # Group (mp0,h0) runs while its stream lands.  Its first output
        # DMA is the first gpsimd-queue instruction and depends on the
        # eviction tile, so the gpsimd-queue q/r DMAs for m-pairs 1..3
        # (issued right after) only start once the JIT load phase is
        # over and don't steal HBM bandwidth from it.
        run_group(0, 0)
        for mp in range(1, 4):
            for kd in range(KD):
                load_q(kd, mp, nc.gpsimd)
                if kd < H_CORR:
                    load_r(kd, mp, nc.gpsimd)
        for mp, h in [(0, 1), (1, 0), (1, 1), (2, 0), (2, 1), (3, 0)]:
            run_group(mp, h)
        # Final group in two 4-chain halves so its eviction + output
        # DMA overlap the remaining matmuls.
        run_group(3, 1, nbs=[4, 5])
        run_group(3, 1, nbs=[6, 7])


def _bf16_body(nc, tc, kxm, kxn, out, mm_dt, mybir):
    """Previous-best bf16 path (x^T resident, sign(w)^T streamed)."""
    P = 128
    KT = D_IN // P
    MT = M_CORE // P
    NW = 512
    NB = D_OUT // NW
    f32 = mybir.dt.float32

    from contextlib import ExitStack
    with ExitStack() as ctx:
        kxm_pool = ctx.enter_context(tc.tile_pool(name="kxm", bufs=1))
        kxn_pool = ctx.enter_context(tc.tile_pool(name="kxn", bufs=9))
        psum_pool = ctx.enter_context(
            tc.tile_pool(name="psum", bufs=8, space="PSUM"))
        out_pool = ctx.enter_context(tc.tile_pool(name="outp", bufs=8))

        def issue_chunk(nb, c, k0, sz):
            t = kxn_pool.tile([P, sz, NW], mm_dt, tag="kxn",
                              name=f"kxn_{nb}_{c}", bufs=24)
            src = kxn[k0 * P:(k0 + sz) * P, nb * NW:(nb + 1) * NW]
            nc.sync.dma_start(
                out=t, in_=src.rearrange("(ko ki) n -> ki ko n", ki=P))
            return [t[:, i, :] for i in range(sz)]

        def issue_chunks(nb, sizes):
            rhs, k0 = [], 0
            for c, sz in enumerate(sizes):
                rhs += issue_chunk(nb, c, k0, sz)
                k0 += sz
            return rhs

        kxm_tiles = {}

        def issue_kxm(k, h):
            kt = kxm_pool.tile([P, M_CORE // 2], mm_dt, tag="kxm",
                               name=f"kxm_{k}_{h}", bufs=2 * KT)
            eng = nc.scalar if h == 0 else nc.gpsimd
            eng.dma_start(out=kt[:, :],
                          in_=kxm[k * P:(k + 1) * P,
                                  h * (M_CORE // 2):(h + 1) * (M_CORE // 2)])
            kxm_tiles[(k, h)] = kt

        def lhsT(k, m):
            h, off = divmod(m, MT // 2)
            return kxm_tiles[(k, h)][:, off * P:(off + 1) * P]

        sizes0 = [2, 2, 2, 2, 4, 4, 4, 4, 4, 4]
        rhs0, k0 = [], 0
        issue_kxm(0, 0)
        issue_kxm(1, 0)
        for c, sz in enumerate(sizes0):
            rhs0 += issue_chunk(0, c, k0, sz)
            k0 += sz
            for k in range(min(k0 + 2, KT)):
                if (k, 0) not in kxm_tiles:
                    issue_kxm(k, 0)
            for k in range(min(k0 - 8, KT)):
                if (k, 1) not in kxm_tiles:
                    issue_kxm(k, 1)
        for k in range(KT):
            if (k, 0) not in kxm_tiles:
                issue_kxm(k, 0)
        for k in range(KT):
            if (k, 1) not in kxm_tiles:
                issue_kxm(k, 1)

        next_rhs = rhs0
        for nb in range(NB):
            ncols = slice(nb * NW, (nb + 1) * NW)
            rhs_k = next_rhs
            psums = [psum_pool.tile([P, NW], f32, tag="ps", name=f"ps_{nb}_{i}")
                     for i in range(MT)]
            groups = [range(MT // 2), range(MT // 2, MT)] if nb == 0 \
                else [[m] for m in range(MT)]
            for gi, ms in enumerate(groups):
                for k in range(KT):
                    for m in ms:
                        nc.tensor.matmul(
                            psums[m][:, :],
                            lhsT=lhsT(k, m),
                            rhs=rhs_k[k],
                            start=(k == 0), stop=(k == KT - 1))
                if gi == 0 and nb + 1 < NB:
                    next_rhs = issue_chunks(nb + 1, [4] * 8)
                for m in ms:
                    ot = out_pool.tile([P, NW], f32, tag="ot", name=f"ot_{nb}_{m}")
                    nc.vector.tensor_copy(out=ot[:, :], in_=psums[m][:, :])
                    nc.gpsimd.dma_start(
                        out=out[m * P:(m + 1) * P, ncols], in_=ot[:, :])


def _build():
    """Build + compile the 8-core SPMD Bass program once per process."""
    if "nc" in _cache:
        return _cache["nc"]

    import concourse.bacc as bacc
    import concourse.tile as tile
    import concourse.mybir as mybir
    from concourse.kernels.tile_matmul import matmul_tile_kernel

    mm_dt = {"f32r": mybir.dt.float32r, "bf16": mybir.dt.bfloat16,
             "fp8e4": mybir.dt.float8e4}[DTYPE]

    nc = bacc.Bacc("TRN2", target_bir_lowering=False, debug=False,
                   enable_asserts=bool(os.environ.get("BK_ASSERTS")),
                   num_devices=NCORES)
    kxr = None
    if DTYPE == "fp8e4" and IMPL == "custom":
        # Pre-tiled layouts (see _prep_inputs): row index is
        # (tile_index * 128 + partition), columns are the tile's
        # contiguous (ko, free) payload.
        kxm = nc.dram_tensor("kxm", [4 * 16 * 128, 512], mm_dt,
                             kind="ExternalInput").ap()
        kxn = nc.dram_tensor("kxn", [2 * 16 * 128, 4096], mm_dt,
                             kind="ExternalInput").ap()
        kxr = nc.dram_tensor("kxr", [4 * max(H_CORR, 1) * 128, 512], mm_dt,
                             kind="ExternalInput").ap()
    else:
        kxm = nc.dram_tensor("kxm", [D_IN, M_CORE], mm_dt,
                             kind="ExternalInput").ap()
        kxn = nc.dram_tensor("kxn", [D_IN, D_OUT], mm_dt,
                             kind="ExternalInput").ap()
    out = nc.dram_tensor("out", [M_CORE, D_OUT], mybir.dt.float32,
                         kind="ExternalOutput").ap()

    def _warmup(tc):
        # The PE clock is HAM-throttled to 1.2GHz until ~3.4us of
        # sustained matmul activity. Burn the initial DMA window
        # warming the clock gate; the PSUM bank frees on pool exit.
        from contextlib import ExitStack
        with ExitStack() as ctx:
            wp = ctx.enter_context(tc.tile_pool(name="warm", bufs=1))
            wpp = ctx.enter_context(
                tc.tile_pool(name="warmp", bufs=1, space="PSUM"))
            wdt = mybir.dt.bfloat16
            a = wp.tile([128, 128], wdt)
            b = wp.tile([128, 512], wdt)
            nc.any.memset(a[:, :], 0.0)
            nc.any.memset(b[:, :], 0.0)
            ps = wpp.tile([128, 512], mybir.dt.float32)
            for _ in range(int(os.environ.get("BK_WARM", "20"))):
                nc.tensor.matmul(ps[:, :], lhsT=a[:, :], rhs=b[:, :],
                                 start=True, stop=True)

    if IMPL == "custom" and DTYPE == "fp8e4":
        with tile.TileContext(nc) as tc:
            _warmup(tc)
            _dr_body(nc, tc, kxm, kxr, kxn, out, mybir)
    elif IMPL == "custom":
        with tile.TileContext(nc) as tc:
            _warmup(tc)
            _bf16_body(nc, tc, kxm, kxn, out, mm_dt, mybir)
    else:
        kw = {}
        if os.environ.get("BK_MAX_K_TILE"):
            kw["MAX_K_TILE_SIZE"] = int(os.environ["BK_MAX_K_TILE"])
        if os.environ.get("BK_SKIP_K_SNAKE"):
            kw["skip_k_snake"] = True
        if os.environ.get("BK_NO_CACHE_TILES"):
            kw["cache_tiles"] = False
        with tile.TileContext(nc) as tc:
            _warmup(tc)
            matmul_tile_kernel(tc, kxm, kxn, out, **kw)
    nc.compile()
    _cache["nc"] = nc
    return nc


def _prep_inputs(x, weight):
    import ml_dtypes
    if DTYPE == "bf16":
        np_dt = ml_dtypes.bfloat16
    elif DTYPE == "fp8e4":
        np_dt = ml_dtypes.float8_e4m3
    else:
        np_dt = np.float32
    x2d = np.asarray(x, dtype=np.float32).reshape(M_TOTAL, D_IN)
    sgn = np.sign(weight, dtype=np.float32).T.astype(np_dt)

    def tile5(a, nt, free):
        # [nt*256, M] -> [mp, kd, p, ko, mpw] -> [(mp kd p), 512]
        kd_n = a.shape[0] // 256
        t = a.reshape(kd_n, 2, 128, 4, free).transpose(3, 0, 2, 1, 4)
        return np.ascontiguousarray(t).reshape(4 * kd_n * 128, 2 * free)

    if DTYPE == "fp8e4" and IMPL == "custom":
        # W: [h, kd, p, ko, 2048] -> [(h kd p), 4096]
        wt = sgn.reshape(16, 2, 128, 2, 2048).transpose(3, 0, 2, 1, 4)
        kxn = np.ascontiguousarray(wt).reshape(2 * 16 * 128, 4096)
    else:
        kxn = np.ascontiguousarray(sgn)
    in_maps = []
    for c in range(NCORES):
        xs = x2d[c * M_CORE:(c + 1) * M_CORE].T  # [D_IN, M_CORE]
        q8 = xs.astype(np_dt)
        if DTYPE == "fp8e4" and IMPL == "custom":
            kc = max(H_CORR, 1) * 256
            resid = (xs[:kc] - q8[:kc].astype(np.float32)).astype(np_dt)
            if H_CORR == 0:
                resid[:] = 0
            im = {"kxm": tile5(q8, 16, 256),
                  "kxr": tile5(resid, max(H_CORR, 1), 256),
                  "kxn": kxn}
        else:
            im = {"kxm": np.ascontiguousarray(q8), "kxn": kxn}
        in_maps.append(im)
    return in_maps


def _run(x, weight, bias, trace=False):
    from concourse.bass_utils import run_bass_kernel_spmd

    nc = _build()
    in_maps = _prep_inputs(x, weight)
    res = run_bass_kernel_spmd(nc, in_maps, core_ids=list(range(NCORES)),
                               trace=trace)
    out = np.concatenate([res.results[c]["out"] for c in range(NCORES)],
                         axis=0)
    bias = np.asarray(bias, dtype=np.float32)
    if np.any(bias):
        out += bias
    return out.reshape(B, S, D_OUT), res


def kernel(x, weight, bias):
    out, _ = _run(x, weight, bias, trace=False)
    return out


# revision 20
# speedup vs baseline: 1.6022x; 1.0343x over previous
"""BitNet-style row-parallel linear on 8 TRN2 NeuronCores.

Reference computes: out[b,s,o] = sum_d x[b,s,d] * sign(w[o,d]) + bias[o]
  x: [4, 2048, 4096] f32, w: [4096, 4096] f32, bias: [4096] f32.

Strategy: data-parallel over the 8192 (b*s) rows — each of the 8 cores
computes a 1024-row slice of the output against the full binarized
weight. No collective needed; shards concatenate to the full output.
(The row-parallel/all-reduce hint costs a 128MB all-reduce per core;
sharding M instead makes the partial outputs disjoint.)

Dtype: fp8 e4m3 with DoubleRow perf mode (2 fp8 weights per PE cell,
2 MACs/cycle -> ~1.8x the bf16 ALU rate). Weights sign(w) are exactly
representable in e4m3; x quantization alone gives rel err 0.0212
(> 2e-2 gate), so the first H_CORR of the 16 double-row k-groups also
accumulate an e4m3 residual r = e4m3(x - e4m3(x)) against the same
weight tiles (residual magnitude <= |x|/16 fits e4m3 directly, no
rescale needed). H_CORR=4 gives exact rel err 0.0185 on the reference
inputs; bias from the reference is zero but still applied on host.

Everything (x^T 4.2MB, sign(w)^T 16.8MB, residual 1MB in e4m3) is
SBUF-resident per core, so after the initial DMA the 1280 matmuls run
back-to-back with no HBM traffic except output eviction.
"""

import os
import numpy as np

B, S, D_IN, D_OUT = 4, 2048, 4096, 4096
NCORES = 8
M_TOTAL = B * S
M_CORE = M_TOTAL // NCORES

_cache = {}

# "fp8e4" (DoubleRow + residual correction), "bf16", or "f32r".
DTYPE = os.environ.get("BK_DTYPE", "fp8e4")
IMPL = os.environ.get("BK_IMPL", "custom")
# Number of double-row k-groups (256 k each) that get the residual
# correction pass. 4 -> rel err 0.0185, 6 -> 0.0159, 0 -> 0.0212.
H_CORR = int(os.environ.get("BK_HCORR", "4"))


def _dr_body(nc, tc, kxm, kxr, kxn, out, mybir):
    """fp8 DoubleRow matmul with residual-corrected accumulation.

    All operands SBUF-resident, loaded in a DMA order matched to the
    compute schedule so the PE never waits on HBM:
      - W (sign(w)^T) splits into 32 tiles (16 double-k groups x 2
        n-halves, 0.5MB each): n-half 0 streams first.
      - q (x^T) and r (residual) split by m-pair (64KB tiles) so the
        first chain group's per-k DMA need (0.56MB) arrives faster
        than its 8 matmuls (1.73us) execute.
    Chain groups: 8 PSUM banks hold the accumulation chains of one
    (m-pair x n-half) group = 2m x 4nb; groups run in an order whose
    W/q needs track the DMA stream.  Each chain is 16 main + H_CORR
    residual DoubleRow matmuls; corrections interleave right after
    their main k-group (their W tile is then already resident).
    """
    P = 128
    KD = D_IN // 256          # 16 double-row k groups
    MT = M_CORE // P          # 8 m tiles
    NW = 512
    NB = D_OUT // NW          # 8 n blocks
    NHALF = 2048
    MP = 256                  # m-pair width (2 m tiles)
    f32 = mybir.dt.float32
    fp8 = mybir.dt.float8e4
    DR = mybir.MatmulPerfMode.DoubleRow

    from contextlib import ExitStack
    with ExitStack() as ctx:
        w_pool = ctx.enter_context(tc.tile_pool(name="w", bufs=1))
        q_pool = ctx.enter_context(tc.tile_pool(name="q", bufs=1))
        r_pool = ctx.enter_context(tc.tile_pool(name="r", bufs=1))
        psum_pool = ctx.enter_context(
            tc.tile_pool(name="psum", bufs=8, space="PSUM"))
        out_pool = ctx.enter_context(tc.tile_pool(name="outp", bufs=8))

        w_tiles = {}   # (kd, h) -> [P, 2, NHALF]
        q_tiles = {}   # (kd, mp) -> [P, 2, MP]
        r_tiles = {}   # (kc, mp) -> [P, 2, MP]

        # DRAM is host-pre-tiled so every DMA is a contiguous block
        # with >=512B per-partition lines (W: 4KB) and no gather cost.
        def load_w(kd, h, eng):
            wt = w_pool.tile([P, 2, NHALF], fp8, tag="w",
                             name=f"w_{kd}_{h}", bufs=2 * KD)
            base = (h * KD + kd) * P
            eng.dma_start(
                out=wt, in_=kxn[base:base + P, :].rearrange(
                    "p (ko n) -> p ko n", ko=2))
            w_tiles[(kd, h)] = wt

        def load_q(kd, mp, eng):
            qt = q_pool.tile([P, 2, MP], fp8, tag="q",
                             name=f"q_{kd}_{mp}", bufs=4 * KD)
            base = (mp * KD + kd) * P
            eng.dma_start(
                out=qt, in_=kxm[base:base + P, :].rearrange(
                    "p (ko m) -> p ko m", ko=2))
            q_tiles[(kd, mp)] = qt

        def load_r(kc, mp, eng):
            rt = r_pool.tile([P, 2, MP], fp8, tag="r",
                             name=f"r_{kc}_{mp}", bufs=4 * max(H_CORR, 1))
            base = (mp * max(H_CORR, 1) + kc) * P
            eng.dma_start(
                out=rt, in_=kxr[base:base + P, :].rearrange(
                    "p (ko m) -> p ko m", ko=2))
            r_tiles[(kc, mp)] = rt

        # Sync queue carries a single just-in-time stream matched to
        # group (mp0,h0)'s consumption: per k-group its q (and r)
        # slices land right before its 0.5MB W tile, then W n-half 1
        # follows for group (mp0,h1).  q/r of m-pairs 1..3 go on the
        # gpsimd queue, which is gated below so they don't steal HBM
        # bandwidth from this stream.
        for kd in range(KD):
            load_q(kd, 0, nc.sync)
            if kd < H_CORR:
                load_r(kd, 0, nc.sync)
            load_w(kd, 0, nc.sync)
        for kd in range(KD):
            load_w(kd, 1, nc.sync)

        # Chain steps: main k-group kd, with its residual correction
        # right after for kd < H_CORR.
        steps = []
        for kd in range(KD):
            steps.append((q_tiles, kd))
            if kd < H_CORR:
                steps.append((r_tiles, kd))

        def run_group(mp, h, nbs=None):
            """Accumulation chains for (m-pair mp) x (n-blocks of half h)."""
            nbs = range(h * 4, h * 4 + 4) if nbs is None else nbs
            chains = [(m, nb) for m in (2 * mp, 2 * mp + 1) for nb in nbs]
            psums = {c: psum_pool.tile([P, NW], f32, tag="ps",
                                       name=f"ps_{c[0]}_{c[1]}")
                     for c in chains}
            for si, (tiles, kd) in enumerate(steps):
                for m, nb in chains:
                    nc.tensor.matmul(
                        psums[(m, nb)][:, :],
                        lhsT=tiles[(kd, mp)][:, :, (m % 2) * P:(m % 2 + 1) * P],
                        rhs=w_tiles[(kd, h)][:, :,
                                             (nb - h * 4) * NW:
                                             (nb - h * 4 + 1) * NW],
                        start=(si == 0), stop=(si == len(steps) - 1),
                        perf_mode=DR)
            for j, (m, nb) in enumerate(chains):
                ot = out_pool.tile([P, NW], f32, tag="ot",
                                   name=f"ot_{m}_{nb}")
                nc.vector.tensor_copy(out=ot[:, :], in_=psums[(m, nb)][:, :])
                if len(chains) < 8:
                    # tail chunks: sync queue is idle by then
                    eng = (nc.sync, nc.scalar, nc.gpsimd)[j % 3]
                else:
                    eng = nc.scalar if j % 2 else nc.gpsimd
                eng.dma_start(
                    out=out[m * P:(m + 1) * P, nb * NW:(nb + 1) * NW],
                    in_=ot[:, :])

        # Group (mp0,h0) runs while its stream lands.  Its first output
        # DMA is the first gpsimd-queue instruction and depends on the
        # eviction tile, so the gpsimd-queue q/r DMAs for m-pairs 1..3
        # (issued right after) only start once the JIT load phase is
        # over and don't steal HBM bandwidth from it.
        run_group(0, 0)
        for mp in range(1, 4):
            for kd in range(KD):
                load_q(kd, mp, nc.gpsimd)
                if kd < H_CORR:
                    load_r(kd, mp, nc.gpsimd)
        for mp, h in [(0, 1), (1, 0), (1, 1), (2, 0), (2, 1), (3, 0)]:
            run_group(mp, h)
        # Final group in shrinking chunks so its eviction + output
        # DMA overlap the remaining matmuls and the tail is short.
        run_group(3, 1, nbs=[4, 5])
        run_group(3, 1, nbs=[6])
        run_group(3, 1, nbs=[7])


def _bf16_body(nc, tc, kxm, kxn, out, mm_dt, mybir):
    """Previous-best bf16 path (x^T resident, sign(w)^T streamed)."""
    P = 128
    KT = D_IN // P
    MT = M_CORE // P
    NW = 512
    NB = D_OUT // NW
    f32 = mybir.dt.float32

    from contextlib import ExitStack
    with ExitStack() as ctx:
        kxm_pool = ctx.enter_context(tc.tile_pool(name="kxm", bufs=1))
        kxn_pool = ctx.enter_context(tc.tile_pool(name="kxn", bufs=9))
        psum_pool = ctx.enter_context(
            tc.tile_pool(name="psum", bufs=8, space="PSUM"))
        out_pool = ctx.enter_context(tc.tile_pool(name="outp", bufs=8))

        def issue_chunk(nb, c, k0, sz):
            t = kxn_pool.tile([P, sz, NW], mm_dt, tag="kxn",
                              name=f"kxn_{nb}_{c}", bufs=24)
            src = kxn[k0 * P:(k0 + sz) * P, nb * NW:(nb + 1) * NW]
            nc.sync.dma_start(
                out=t, in_=src.rearrange("(ko ki) n -> ki ko n", ki=P))
            return [t[:, i, :] for i in range(sz)]

        def issue_chunks(nb, sizes):
            rhs, k0 = [], 0
            for c, sz in enumerate(sizes):
                rhs += issue_chunk(nb, c, k0, sz)
                k0 += sz
            return rhs

        kxm_tiles = {}

        def issue_kxm(k, h):
            kt = kxm_pool.tile([P, M_CORE // 2], mm_dt, tag="kxm",
                               name=f"kxm_{k}_{h}", bufs=2 * KT)
            eng = nc.scalar if h == 0 else nc.gpsimd
            eng.dma_start(out=kt[:, :],
                          in_=kxm[k * P:(k + 1) * P,
                                  h * (M_CORE // 2):(h + 1) * (M_CORE // 2)])
            kxm_tiles[(k, h)] = kt

        def lhsT(k, m):
            h, off = divmod(m, MT // 2)
            return kxm_tiles[(k, h)][:, off * P:(off + 1) * P]

        sizes0 = [2, 2, 2, 2, 4, 4, 4, 4, 4, 4]
        rhs0, k0 = [], 0
        issue_kxm(0, 0)
        issue_kxm(1, 0)
        for c, sz in enumerate(sizes0):
            rhs0 += issue_chunk(0, c, k0, sz)
            k0 += sz
            for k in range(min(k0 + 2, KT)):
                if (k, 0) not in kxm_tiles:
                    issue_kxm(k, 0)
            for k in range(min(k0 - 8, KT)):
                if (k, 1) not in kxm_tiles:
                    issue_kxm(k, 1)
        for k in range(KT):
            if (k, 0) not in kxm_tiles:
                issue_kxm(k, 0)
        for k in range(KT):
            if (k, 1) not in kxm_tiles:
                issue_kxm(k, 1)

        next_rhs = rhs0
        for nb in range(NB):
            ncols = slice(nb * NW, (nb + 1) * NW)
            rhs_k = next_rhs
            psums = [psum_pool.tile([P, NW], f32, tag="ps", name=f"ps_{nb}_{i}")
                     for i in range(MT)]
            groups = [range(MT // 2), range(MT // 2, MT)] if nb == 0 \
                else [[m] for m in range(MT)]
            for gi, ms in enumerate(groups):
                for k in range(KT):
                    for m in ms:
                        nc.tensor.matmul(
                            psums[m][:, :],
                            lhsT=lhsT(k, m),
                            rhs=rhs_k[k],
                            start=(k == 0), stop=(k == KT - 1))
                if gi == 0 and nb + 1 < NB:
                    next_rhs = issue_chunks(nb + 1, [4] * 8)
                for m in ms:
                    ot = out_pool.tile([P, NW], f32, tag="ot", name=f"ot_{nb}_{m}")
                    nc.vector.tensor_copy(out=ot[:, :], in_=psums[m][:, :])
                    nc.gpsimd.dma_start(
                        out=out[m * P:(m + 1) * P, ncols], in_=ot[:, :])


def _build():
    """Build + compile the 8-core SPMD Bass program once per process."""
    if "nc" in _cache:
        return _cache["nc"]

    import concourse.bacc as bacc
    import concourse.tile as tile
    import concourse.mybir as mybir
    from concourse.kernels.tile_matmul import matmul_tile_kernel

    mm_dt = {"f32r": mybir.dt.float32r, "bf16": mybir.dt.bfloat16,
             "fp8e4": mybir.dt.float8e4}[DTYPE]

    nc = bacc.Bacc("TRN2", target_bir_lowering=False, debug=False,
                   enable_asserts=bool(os.environ.get("BK_ASSERTS")),
                   num_devices=NCORES)
    kxr = None
    if DTYPE == "fp8e4" and IMPL == "custom":
        # Pre-tiled layouts (see _prep_inputs): row index is
        # (tile_index * 128 + partition), columns are the tile's
        # contiguous (ko, free) payload.
        kxm = nc.dram_tensor("kxm", [4 * 16 * 128, 512], mm_dt,
                             kind="ExternalInput").ap()
        kxn = nc.dram_tensor("kxn", [2 * 16 * 128, 4096], mm_dt,
                             kind="ExternalInput").ap()
        kxr = nc.dram_tensor("kxr", [4 * max(H_CORR, 1) * 128, 512], mm_dt,
                             kind="ExternalInput").ap()
    else:
        kxm = nc.dram_tensor("kxm", [D_IN, M_CORE], mm_dt,
                             kind="ExternalInput").ap()
        kxn = nc.dram_tensor("kxn", [D_IN, D_OUT], mm_dt,
                             kind="ExternalInput").ap()
    out = nc.dram_tensor("out", [M_CORE, D_OUT], mybir.dt.float32,
                         kind="ExternalOutput").ap()

    def _warmup(tc):
        # The PE clock is HAM-throttled to 1.2GHz until ~3.4us of
        # sustained matmul activity. Burn the initial DMA window
        # warming the clock gate; the PSUM bank frees on pool exit.
        from contextlib import ExitStack
        with ExitStack() as ctx:
            wp = ctx.enter_context(tc.tile_pool(name="warm", bufs=1))
            wpp = ctx.enter_context(
                tc.tile_pool(name="warmp", bufs=1, space="PSUM"))
            wdt = mybir.dt.bfloat16
            a = wp.tile([128, 128], wdt)
            b = wp.tile([128, 512], wdt)
            nc.any.memset(a[:, :], 0.0)
            nc.any.memset(b[:, :], 0.0)
            ps = wpp.tile([128, 512], mybir.dt.float32)
            for _ in range(int(os.environ.get("BK_WARM", "10"))):
                nc.tensor.matmul(ps[:, :], lhsT=a[:, :], rhs=b[:, :],
                                 start=True, stop=True)

    if IMPL == "custom" and DTYPE == "fp8e4":
        with tile.TileContext(nc) as tc:
            _warmup(tc)
            _dr_body(nc, tc, kxm, kxr, kxn, out, mybir)
    elif IMPL == "custom":
        with tile.TileContext(nc) as tc:
            _warmup(tc)
            _bf16_body(nc, tc, kxm, kxn, out, mm_dt, mybir)
    else:
        kw = {}
        if os.environ.get("BK_MAX_K_TILE"):
            kw["MAX_K_TILE_SIZE"] = int(os.environ["BK_MAX_K_TILE"])
        if os.environ.get("BK_SKIP_K_SNAKE"):
            kw["skip_k_snake"] = True
        if os.environ.get("BK_NO_CACHE_TILES"):
            kw["cache_tiles"] = False
        with tile.TileContext(nc) as tc:
            _warmup(tc)
            matmul_tile_kernel(tc, kxm, kxn, out, **kw)
    nc.compile()
    _cache["nc"] = nc
    return nc


def _prep_inputs(x, weight):
    import ml_dtypes
    if DTYPE == "bf16":
        np_dt = ml_dtypes.bfloat16
    elif DTYPE == "fp8e4":
        np_dt = ml_dtypes.float8_e4m3
    else:
        np_dt = np.float32
    x2d = np.asarray(x, dtype=np.float32).reshape(M_TOTAL, D_IN)
    sgn = np.sign(weight, dtype=np.float32).T.astype(np_dt)

    def tile5(a, nt, free):
        # [nt*256, M] -> [mp, kd, p, ko, mpw] -> [(mp kd p), 512]
        kd_n = a.shape[0] // 256
        t = a.reshape(kd_n, 2, 128, 4, free).transpose(3, 0, 2, 1, 4)
        return np.ascontiguousarray(t).reshape(4 * kd_n * 128, 2 * free)

    if DTYPE == "fp8e4" and IMPL == "custom":
        # W: [h, kd, p, ko, 2048] -> [(h kd p), 4096]
        wt = sgn.reshape(16, 2, 128, 2, 2048).transpose(3, 0, 2, 1, 4)
        kxn = np.ascontiguousarray(wt).reshape(2 * 16 * 128, 4096)
    else:
        kxn = np.ascontiguousarray(sgn)
    in_maps = []
    for c in range(NCORES):
        xs = x2d[c * M_CORE:(c + 1) * M_CORE].T  # [D_IN, M_CORE]
        q8 = xs.astype(np_dt)
        if DTYPE == "fp8e4" and IMPL == "custom":
            kc = max(H_CORR, 1) * 256
            resid = (xs[:kc] - q8[:kc].astype(np.float32)).astype(np_dt)
            if H_CORR == 0:
                resid[:] = 0
            im = {"kxm": tile5(q8, 16, 256),
                  "kxr": tile5(resid, max(H_CORR, 1), 256),
                  "kxn": kxn}
        else:
            im = {"kxm": np.ascontiguousarray(q8), "kxn": kxn}
        in_maps.append(im)
    return in_maps


def _run(x, weight, bias, trace=False):
    from concourse.bass_utils import run_bass_kernel_spmd

    nc = _build()
    in_maps = _prep_inputs(x, weight)
    res = run_bass_kernel_spmd(nc, in_maps, core_ids=list(range(NCORES)),
                               trace=trace)
    out = np.concatenate([res.results[c]["out"] for c in range(NCORES)],
                         axis=0)
    bias = np.asarray(bias, dtype=np.float32)
    if np.any(bias):
        out += bias
    return out.reshape(B, S, D_OUT), res


def kernel(x, weight, bias):
    out, _ = _run(x, weight, bias, trace=False)
    return out


# revision 21
# speedup vs baseline: 1.6090x; 1.0043x over previous
"""BitNet-style row-parallel linear on 8 TRN2 NeuronCores.

Reference computes: out[b,s,o] = sum_d x[b,s,d] * sign(w[o,d]) + bias[o]
  x: [4, 2048, 4096] f32, w: [4096, 4096] f32, bias: [4096] f32.

Strategy: data-parallel over the 8192 (b*s) rows — each of the 8 cores
computes a 1024-row slice of the output against the full binarized
weight. No collective needed; shards concatenate to the full output.
(The row-parallel/all-reduce hint costs a 128MB all-reduce per core;
sharding M instead makes the partial outputs disjoint.)

Dtype: fp8 e4m3 with DoubleRow perf mode (2 fp8 weights per PE cell,
2 MACs/cycle -> ~1.8x the bf16 ALU rate). Weights sign(w) are exactly
representable in e4m3; x quantization alone gives rel err 0.0212
(> 2e-2 gate), so the first H_CORR of the 16 double-row k-groups also
accumulate an e4m3 residual r = e4m3(x - e4m3(x)) against the same
weight tiles (residual magnitude <= |x|/16 fits e4m3 directly, no
rescale needed). H_CORR=4 gives exact rel err 0.0185 on the reference
inputs; bias from the reference is zero but still applied on host.

Everything (x^T 4.2MB, sign(w)^T 16.8MB, residual 1MB in e4m3) is
SBUF-resident per core, so after the initial DMA the 1280 matmuls run
back-to-back with no HBM traffic except output eviction.
"""

import os
import numpy as np

B, S, D_IN, D_OUT = 4, 2048, 4096, 4096
NCORES = 8
M_TOTAL = B * S
M_CORE = M_TOTAL // NCORES

_cache = {}

# "fp8e4" (DoubleRow + residual correction), "bf16", or "f32r".
DTYPE = os.environ.get("BK_DTYPE", "fp8e4")
IMPL = os.environ.get("BK_IMPL", "custom")
# Number of double-row k-groups (256 k each) that get the residual
# correction pass. 4 -> rel err 0.0185, 6 -> 0.0159, 0 -> 0.0212.
H_CORR = int(os.environ.get("BK_HCORR", "4"))


def _dr_body(nc, tc, kxm, kxr, kxn, out, mybir):
    """fp8 DoubleRow matmul with residual-corrected accumulation.

    All operands SBUF-resident, loaded in a DMA order matched to the
    compute schedule so the PE never waits on HBM:
      - W (sign(w)^T) splits into 32 tiles (16 double-k groups x 2
        n-halves, 0.5MB each): n-half 0 streams first.
      - q (x^T) and r (residual) split by m-pair (64KB tiles) so the
        first chain group's per-k DMA need (0.56MB) arrives faster
        than its 8 matmuls (1.73us) execute.
    Chain groups: 8 PSUM banks hold the accumulation chains of one
    (m-pair x n-half) group = 2m x 4nb; groups run in an order whose
    W/q needs track the DMA stream.  Each chain is 16 main + H_CORR
    residual DoubleRow matmuls; corrections interleave right after
    their main k-group (their W tile is then already resident).
    """
    P = 128
    KD = D_IN // 256          # 16 double-row k groups
    MT = M_CORE // P          # 8 m tiles
    NW = 512
    NB = D_OUT // NW          # 8 n blocks
    NHALF = 2048
    MP = 256                  # m-pair width (2 m tiles)
    f32 = mybir.dt.float32
    fp8 = mybir.dt.float8e4
    DR = mybir.MatmulPerfMode.DoubleRow

    from contextlib import ExitStack
    with ExitStack() as ctx:
        w_pool = ctx.enter_context(tc.tile_pool(name="w", bufs=1))
        q_pool = ctx.enter_context(tc.tile_pool(name="q", bufs=1))
        r_pool = ctx.enter_context(tc.tile_pool(name="r", bufs=1))
        psum_pool = ctx.enter_context(
            tc.tile_pool(name="psum", bufs=8, space="PSUM"))
        out_pool = ctx.enter_context(tc.tile_pool(name="outp", bufs=8))

        w_tiles = {}   # (kd, h) -> [P, 2, NHALF]
        q_tiles = {}   # (kd, mp) -> [P, 2, MP]
        r_tiles = {}   # (kc, mp) -> [P, 2, MP]

        # DRAM is host-pre-tiled so every DMA is a contiguous block
        # with >=512B per-partition lines (W: 4KB) and no gather cost.
        def load_w(kd, h, eng):
            wt = w_pool.tile([P, 2, NHALF], fp8, tag="w",
                             name=f"w_{kd}_{h}", bufs=2 * KD)
            base = (h * KD + kd) * P
            eng.dma_start(
                out=wt, in_=kxn[base:base + P, :].rearrange(
                    "p (ko n) -> p ko n", ko=2))
            w_tiles[(kd, h)] = wt

        def load_q(kd, mp, eng):
            qt = q_pool.tile([P, 2, MP], fp8, tag="q",
                             name=f"q_{kd}_{mp}", bufs=4 * KD)
            base = (mp * KD + kd) * P
            eng.dma_start(
                out=qt, in_=kxm[base:base + P, :].rearrange(
                    "p (ko m) -> p ko m", ko=2))
            q_tiles[(kd, mp)] = qt

        def load_r(kc, mp, eng):
            rt = r_pool.tile([P, 2, MP], fp8, tag="r",
                             name=f"r_{kc}_{mp}", bufs=4 * max(H_CORR, 1))
            base = (mp * max(H_CORR, 1) + kc) * P
            eng.dma_start(
                out=rt, in_=kxr[base:base + P, :].rearrange(
                    "p (ko m) -> p ko m", ko=2))
            r_tiles[(kc, mp)] = rt

        # Sync queue carries a single just-in-time stream matched to
        # group (mp0,h0)'s consumption: per k-group its q (and r)
        # slices land right before its 0.5MB W tile, then W n-half 1
        # follows for group (mp0,h1).  q/r of m-pairs 1..3 go on the
        # gpsimd queue, which is gated below so they don't steal HBM
        # bandwidth from this stream.
        for kd in range(KD):
            load_q(kd, 0, nc.sync)
            if kd < H_CORR:
                load_r(kd, 0, nc.sync)
            load_w(kd, 0, nc.sync)
        for kd in range(KD):
            load_w(kd, 1, nc.sync)

        # Chain steps: main k-group kd, with its residual correction
        # right after for kd < H_CORR.
        steps = []
        for kd in range(KD):
            steps.append((q_tiles, kd))
            if kd < H_CORR:
                steps.append((r_tiles, kd))

        def run_group(mp, h, nbs=None):
            """Accumulation chains for (m-pair mp) x (n-blocks of half h)."""
            nbs = range(h * 4, h * 4 + 4) if nbs is None else nbs
            chains = [(m, nb) for m in (2 * mp, 2 * mp + 1) for nb in nbs]
            psums = {c: psum_pool.tile([P, NW], f32, tag="ps",
                                       name=f"ps_{c[0]}_{c[1]}")
                     for c in chains}
            for si, (tiles, kd) in enumerate(steps):
                for m, nb in chains:
                    nc.tensor.matmul(
                        psums[(m, nb)][:, :],
                        lhsT=tiles[(kd, mp)][:, :, (m % 2) * P:(m % 2 + 1) * P],
                        rhs=w_tiles[(kd, h)][:, :,
                                             (nb - h * 4) * NW:
                                             (nb - h * 4 + 1) * NW],
                        start=(si == 0), stop=(si == len(steps) - 1),
                        perf_mode=DR)
            for j, (m, nb) in enumerate(chains):
                ot = out_pool.tile([P, NW], f32, tag="ot",
                                   name=f"ot_{m}_{nb}")
                nc.vector.tensor_copy(out=ot[:, :], in_=psums[(m, nb)][:, :])
                if len(chains) < 8:
                    # tail chunks: sync queue is idle by then
                    eng = (nc.sync, nc.scalar, nc.gpsimd)[j % 3]
                else:
                    eng = nc.scalar if j % 2 else nc.gpsimd
                eng.dma_start(
                    out=out[m * P:(m + 1) * P, nb * NW:(nb + 1) * NW],
                    in_=ot[:, :])

        # Group (mp0,h0) runs while its stream lands.  Its first output
        # DMA is the first gpsimd-queue instruction and depends on the
        # eviction tile, so the gpsimd-queue q/r DMAs for m-pairs 1..3
        # (issued right after) only start once the JIT load phase is
        # over and don't steal HBM bandwidth from it.
        run_group(0, 0)
        # tile_wait_until pushes these loads' scheduler priority past
        # G0's end so the (FIFO) gpsimd queue really orders them after
        # G0's first output DMA — without it the scheduler hoists them
        # to t=0 and they steal HBM bandwidth from the JIT stream.
        with tc.tile_wait_until(0.030):
            for mp in range(1, 4):
                for kd in range(KD):
                    load_q(kd, mp, nc.gpsimd)
                    if kd < H_CORR:
                        load_r(kd, mp, nc.gpsimd)
        for mp, h in [(0, 1), (1, 0), (1, 1), (2, 0), (2, 1), (3, 0)]:
            run_group(mp, h)
        # Final group in shrinking chunks so its eviction + output
        # DMA overlap the remaining matmuls and the tail is short.
        run_group(3, 1, nbs=[4, 5])
        run_group(3, 1, nbs=[6])
        run_group(3, 1, nbs=[7])


def _bf16_body(nc, tc, kxm, kxn, out, mm_dt, mybir):
    """Previous-best bf16 path (x^T resident, sign(w)^T streamed)."""
    P = 128
    KT = D_IN // P
    MT = M_CORE // P
    NW = 512
    NB = D_OUT // NW
    f32 = mybir.dt.float32

    from contextlib import ExitStack
    with ExitStack() as ctx:
        kxm_pool = ctx.enter_context(tc.tile_pool(name="kxm", bufs=1))
        kxn_pool = ctx.enter_context(tc.tile_pool(name="kxn", bufs=9))
        psum_pool = ctx.enter_context(
            tc.tile_pool(name="psum", bufs=8, space="PSUM"))
        out_pool = ctx.enter_context(tc.tile_pool(name="outp", bufs=8))

        def issue_chunk(nb, c, k0, sz):
            t = kxn_pool.tile([P, sz, NW], mm_dt, tag="kxn",
                              name=f"kxn_{nb}_{c}", bufs=24)
            src = kxn[k0 * P:(k0 + sz) * P, nb * NW:(nb + 1) * NW]
            nc.sync.dma_start(
                out=t, in_=src.rearrange("(ko ki) n -> ki ko n", ki=P))
            return [t[:, i, :] for i in range(sz)]

        def issue_chunks(nb, sizes):
            rhs, k0 = [], 0
            for c, sz in enumerate(sizes):
                rhs += issue_chunk(nb, c, k0, sz)
                k0 += sz
            return rhs

        kxm_tiles = {}

        def issue_kxm(k, h):
            kt = kxm_pool.tile([P, M_CORE // 2], mm_dt, tag="kxm",
                               name=f"kxm_{k}_{h}", bufs=2 * KT)
            eng = nc.scalar if h == 0 else nc.gpsimd
            eng.dma_start(out=kt[:, :],
                          in_=kxm[k * P:(k + 1) * P,
                                  h * (M_CORE // 2):(h + 1) * (M_CORE // 2)])
            kxm_tiles[(k, h)] = kt

        def lhsT(k, m):
            h, off = divmod(m, MT // 2)
            return kxm_tiles[(k, h)][:, off * P:(off + 1) * P]

        sizes0 = [2, 2, 2, 2, 4, 4, 4, 4, 4, 4]
        rhs0, k0 = [], 0
        issue_kxm(0, 0)
        issue_kxm(1, 0)
        for c, sz in enumerate(sizes0):
            rhs0 += issue_chunk(0, c, k0, sz)
            k0 += sz
            for k in range(min(k0 + 2, KT)):
                if (k, 0) not in kxm_tiles:
                    issue_kxm(k, 0)
            for k in range(min(k0 - 8, KT)):
                if (k, 1) not in kxm_tiles:
                    issue_kxm(k, 1)
        for k in range(KT):
            if (k, 0) not in kxm_tiles:
                issue_kxm(k, 0)
        for k in range(KT):
            if (k, 1) not in kxm_tiles:
                issue_kxm(k, 1)

        next_rhs = rhs0
        for nb in range(NB):
            ncols = slice(nb * NW, (nb + 1) * NW)
            rhs_k = next_rhs
            psums = [psum_pool.tile([P, NW], f32, tag="ps", name=f"ps_{nb}_{i}")
                     for i in range(MT)]
            groups = [range(MT // 2), range(MT // 2, MT)] if nb == 0 \
                else [[m] for m in range(MT)]
            for gi, ms in enumerate(groups):
                for k in range(KT):
                    for m in ms:
                        nc.tensor.matmul(
                            psums[m][:, :],
                            lhsT=lhsT(k, m),
                            rhs=rhs_k[k],
                            start=(k == 0), stop=(k == KT - 1))
                if gi == 0 and nb + 1 < NB:
                    next_rhs = issue_chunks(nb + 1, [4] * 8)
                for m in ms:
                    ot = out_pool.tile([P, NW], f32, tag="ot", name=f"ot_{nb}_{m}")
                    nc.vector.tensor_copy(out=ot[:, :], in_=psums[m][:, :])
                    nc.gpsimd.dma_start(
                        out=out[m * P:(m + 1) * P, ncols], in_=ot[:, :])


def _build():
    """Build + compile the 8-core SPMD Bass program once per process."""
    if "nc" in _cache:
        return _cache["nc"]

    import concourse.bacc as bacc
    import concourse.tile as tile
    import concourse.mybir as mybir
    from concourse.kernels.tile_matmul import matmul_tile_kernel

    mm_dt = {"f32r": mybir.dt.float32r, "bf16": mybir.dt.bfloat16,
             "fp8e4": mybir.dt.float8e4}[DTYPE]

    nc = bacc.Bacc("TRN2", target_bir_lowering=False, debug=False,
                   enable_asserts=bool(os.environ.get("BK_ASSERTS")),
                   num_devices=NCORES)
    kxr = None
    if DTYPE == "fp8e4" and IMPL == "custom":
        # Pre-tiled layouts (see _prep_inputs): row index is
        # (tile_index * 128 + partition), columns are the tile's
        # contiguous (ko, free) payload.
        kxm = nc.dram_tensor("kxm", [4 * 16 * 128, 512], mm_dt,
                             kind="ExternalInput").ap()
        kxn = nc.dram_tensor("kxn", [2 * 16 * 128, 4096], mm_dt,
                             kind="ExternalInput").ap()
        kxr = nc.dram_tensor("kxr", [4 * max(H_CORR, 1) * 128, 512], mm_dt,
                             kind="ExternalInput").ap()
    else:
        kxm = nc.dram_tensor("kxm", [D_IN, M_CORE], mm_dt,
                             kind="ExternalInput").ap()
        kxn = nc.dram_tensor("kxn", [D_IN, D_OUT], mm_dt,
                             kind="ExternalInput").ap()
    out = nc.dram_tensor("out", [M_CORE, D_OUT], mybir.dt.float32,
                         kind="ExternalOutput").ap()

    def _warmup(tc):
        # The PE clock is HAM-throttled to 1.2GHz until ~3.4us of
        # sustained matmul activity. Burn the initial DMA window
        # warming the clock gate; the PSUM bank frees on pool exit.
        from contextlib import ExitStack
        with ExitStack() as ctx:
            wp = ctx.enter_context(tc.tile_pool(name="warm", bufs=1))
            wpp = ctx.enter_context(
                tc.tile_pool(name="warmp", bufs=1, space="PSUM"))
            wdt = mybir.dt.bfloat16
            a = wp.tile([128, 128], wdt)
            b = wp.tile([128, 512], wdt)
            nc.any.memset(a[:, :], 0.0)
            nc.any.memset(b[:, :], 0.0)
            ps = wpp.tile([128, 512], mybir.dt.float32)
            for _ in range(int(os.environ.get("BK_WARM", "10"))):
                nc.tensor.matmul(ps[:, :], lhsT=a[:, :], rhs=b[:, :],
                                 start=True, stop=True)

    if IMPL == "custom" and DTYPE == "fp8e4":
        with tile.TileContext(nc) as tc:
            _warmup(tc)
            _dr_body(nc, tc, kxm, kxr, kxn, out, mybir)
    elif IMPL == "custom":
        with tile.TileContext(nc) as tc:
            _warmup(tc)
            _bf16_body(nc, tc, kxm, kxn, out, mm_dt, mybir)
    else:
        kw = {}
        if os.environ.get("BK_MAX_K_TILE"):
            kw["MAX_K_TILE_SIZE"] = int(os.environ["BK_MAX_K_TILE"])
        if os.environ.get("BK_SKIP_K_SNAKE"):
            kw["skip_k_snake"] = True
        if os.environ.get("BK_NO_CACHE_TILES"):
            kw["cache_tiles"] = False
        with tile.TileContext(nc) as tc:
            _warmup(tc)
            matmul_tile_kernel(tc, kxm, kxn, out, **kw)
    nc.compile()
    _cache["nc"] = nc
    return nc


def _prep_inputs(x, weight):
    import ml_dtypes
    if DTYPE == "bf16":
        np_dt = ml_dtypes.bfloat16
    elif DTYPE == "fp8e4":
        np_dt = ml_dtypes.float8_e4m3
    else:
        np_dt = np.float32
    x2d = np.asarray(x, dtype=np.float32).reshape(M_TOTAL, D_IN)
    sgn = np.sign(weight, dtype=np.float32).T.astype(np_dt)

    def tile5(a, nt, free):
        # [nt*256, M] -> [mp, kd, p, ko, mpw] -> [(mp kd p), 512]
        kd_n = a.shape[0] // 256
        t = a.reshape(kd_n, 2, 128, 4, free).transpose(3, 0, 2, 1, 4)
        return np.ascontiguousarray(t).reshape(4 * kd_n * 128, 2 * free)

    if DTYPE == "fp8e4" and IMPL == "custom":
        # W: [h, kd, p, ko, 2048] -> [(h kd p), 4096]
        wt = sgn.reshape(16, 2, 128, 2, 2048).transpose(3, 0, 2, 1, 4)
        kxn = np.ascontiguousarray(wt).reshape(2 * 16 * 128, 4096)
    else:
        kxn = np.ascontiguousarray(sgn)
    in_maps = []
    for c in range(NCORES):
        xs = x2d[c * M_CORE:(c + 1) * M_CORE].T  # [D_IN, M_CORE]
        q8 = xs.astype(np_dt)
        if DTYPE == "fp8e4" and IMPL == "custom":
            kc = max(H_CORR, 1) * 256
            resid = (xs[:kc] - q8[:kc].astype(np.float32)).astype(np_dt)
            if H_CORR == 0:
                resid[:] = 0
            im = {"kxm": tile5(q8, 16, 256),
                  "kxr": tile5(resid, max(H_CORR, 1), 256),
                  "kxn": kxn}
        else:
            im = {"kxm": np.ascontiguousarray(q8), "kxn": kxn}
        in_maps.append(im)
    return in_maps


def _run(x, weight, bias, trace=False):
    from concourse.bass_utils import run_bass_kernel_spmd

    nc = _build()
    in_maps = _prep_inputs(x, weight)
    res = run_bass_kernel_spmd(nc, in_maps, core_ids=list(range(NCORES)),
                               trace=trace)
    out = np.concatenate([res.results[c]["out"] for c in range(NCORES)],
                         axis=0)
    bias = np.asarray(bias, dtype=np.float32)
    if np.any(bias):
        out += bias
    return out.reshape(B, S, D_OUT), res


def kernel(x, weight, bias):
    out, _ = _run(x, weight, bias, trace=False)
    return out


# revision 22
# speedup vs baseline: 1.6207x; 1.0072x over previous
"""BitNet-style row-parallel linear on 8 TRN2 NeuronCores.

Reference computes: out[b,s,o] = sum_d x[b,s,d] * sign(w[o,d]) + bias[o]
  x: [4, 2048, 4096] f32, w: [4096, 4096] f32, bias: [4096] f32.

Strategy: data-parallel over the 8192 (b*s) rows — each of the 8 cores
computes a 1024-row slice of the output against the full binarized
weight. No collective needed; shards concatenate to the full output.
(The row-parallel/all-reduce hint costs a 128MB all-reduce per core;
sharding M instead makes the partial outputs disjoint.)

Dtype: fp8 e4m3 with DoubleRow perf mode (2 fp8 weights per PE cell,
2 MACs/cycle -> ~1.8x the bf16 ALU rate). Weights sign(w) are exactly
representable in e4m3; x quantization alone gives rel err 0.0212
(> 2e-2 gate), so the first H_CORR of the 16 double-row k-groups also
accumulate an e4m3 residual r = e4m3(x - e4m3(x)) against the same
weight tiles (residual magnitude <= |x|/16 fits e4m3 directly, no
rescale needed). H_CORR=4 gives exact rel err 0.0185 on the reference
inputs; bias from the reference is zero but still applied on host.

Everything (x^T 4.2MB, sign(w)^T 16.8MB, residual 1MB in e4m3) is
SBUF-resident per core, so after the initial DMA the 1280 matmuls run
back-to-back with no HBM traffic except output eviction.
"""

import os
import numpy as np

B, S, D_IN, D_OUT = 4, 2048, 4096, 4096
NCORES = 8
M_TOTAL = B * S
M_CORE = M_TOTAL // NCORES

_cache = {}

# "fp8e4" (DoubleRow + residual correction), "bf16", or "f32r".
DTYPE = os.environ.get("BK_DTYPE", "fp8e4")
IMPL = os.environ.get("BK_IMPL", "custom")
# Number of double-row k-groups (256 k each) that get the residual
# correction pass. 4 -> rel err 0.0185, 6 -> 0.0159, 0 -> 0.0212.
H_CORR = int(os.environ.get("BK_HCORR", "4"))


def _dr_body(nc, tc, kxm, kxr, kxn, out, mybir):
    """fp8 DoubleRow matmul with residual-corrected accumulation.

    All operands SBUF-resident, loaded in a DMA order matched to the
    compute schedule so the PE never waits on HBM:
      - W (sign(w)^T) splits into 32 tiles (16 double-k groups x 2
        n-halves, 0.5MB each): n-half 0 streams first.
      - q (x^T) and r (residual) split by m-pair (64KB tiles) so the
        first chain group's per-k DMA need (0.56MB) arrives faster
        than its 8 matmuls (1.73us) execute.
    Chain groups: 8 PSUM banks hold the accumulation chains of one
    (m-pair x n-half) group = 2m x 4nb; groups run in an order whose
    W/q needs track the DMA stream.  Each chain is 16 main + H_CORR
    residual DoubleRow matmuls; corrections interleave right after
    their main k-group (their W tile is then already resident).
    """
    P = 128
    KD = D_IN // 256          # 16 double-row k groups
    MT = M_CORE // P          # 8 m tiles
    NW = 512
    NB = D_OUT // NW          # 8 n blocks
    NHALF = 2048
    MP = 256                  # m-pair width (2 m tiles)
    f32 = mybir.dt.float32
    fp8 = mybir.dt.float8e4
    DR = mybir.MatmulPerfMode.DoubleRow

    from contextlib import ExitStack
    with ExitStack() as ctx:
        w_pool = ctx.enter_context(tc.tile_pool(name="w", bufs=1))
        q_pool = ctx.enter_context(tc.tile_pool(name="q", bufs=1))
        r_pool = ctx.enter_context(tc.tile_pool(name="r", bufs=1))
        psum_pool = ctx.enter_context(
            tc.tile_pool(name="psum", bufs=8, space="PSUM"))
        out_pool = ctx.enter_context(tc.tile_pool(name="outp", bufs=8))

        w_tiles = {}   # (kd, h) -> [P, 2, NHALF]
        q_tiles = {}   # (kd, mp) -> [P, 2, MP]
        r_tiles = {}   # (kc, mp) -> [P, 2, MP]

        # DRAM is host-pre-tiled so every DMA is a contiguous block
        # with >=512B per-partition lines (W: 4KB) and no gather cost.
        def load_w(kd, h, eng):
            wt = w_pool.tile([P, 2, NHALF], fp8, tag="w",
                             name=f"w_{kd}_{h}", bufs=2 * KD)
            base = (h * KD + kd) * P
            eng.dma_start(
                out=wt, in_=kxn[base:base + P, :].rearrange(
                    "p (ko n) -> p ko n", ko=2))
            w_tiles[(kd, h)] = wt

        def load_q(kd, mp, eng):
            qt = q_pool.tile([P, 2, MP], fp8, tag="q",
                             name=f"q_{kd}_{mp}", bufs=4 * KD)
            base = (mp * KD + kd) * P
            eng.dma_start(
                out=qt, in_=kxm[base:base + P, :].rearrange(
                    "p (ko m) -> p ko m", ko=2))
            q_tiles[(kd, mp)] = qt

        def load_r(kc, mp, eng):
            rt = r_pool.tile([P, 2, MP], fp8, tag="r",
                             name=f"r_{kc}_{mp}", bufs=4 * max(H_CORR, 1))
            base = (mp * max(H_CORR, 1) + kc) * P
            eng.dma_start(
                out=rt, in_=kxr[base:base + P, :].rearrange(
                    "p (ko m) -> p ko m", ko=2))
            r_tiles[(kc, mp)] = rt

        # The W stream owns the sync queue (n-half 0 then n-half 1, in
        # group (mp0,h0)/(mp0,h1) consumption order); q/r for m-pair 0
        # stream on the scalar queue.  Separate queues = separate DMA
        # completion counters, so an early matmul's semaphore wait only
        # covers the few triggers it actually needs.  q/r of m-pairs
        # 1..3 go on the gpsimd queue, gated below so they don't steal
        # HBM bandwidth from the JIT phase.
        for kd in range(KD):
            load_q(kd, 0, nc.scalar)
            if kd < H_CORR:
                load_r(kd, 0, nc.scalar)
        for h in range(2):
            for kd in range(KD):
                load_w(kd, h, nc.sync)

        # Chain steps: main k-group kd, with its residual correction
        # right after for kd < H_CORR.
        steps = []
        for kd in range(KD):
            steps.append((q_tiles, kd))
            if kd < H_CORR:
                steps.append((r_tiles, kd))

        def run_group(mp, h, nbs=None):
            """Accumulation chains for (m-pair mp) x (n-blocks of half h)."""
            nbs = range(h * 4, h * 4 + 4) if nbs is None else nbs
            chains = [(m, nb) for m in (2 * mp, 2 * mp + 1) for nb in nbs]
            psums = {c: psum_pool.tile([P, NW], f32, tag="ps",
                                       name=f"ps_{c[0]}_{c[1]}")
                     for c in chains}
            for si, (tiles, kd) in enumerate(steps):
                for m, nb in chains:
                    nc.tensor.matmul(
                        psums[(m, nb)][:, :],
                        lhsT=tiles[(kd, mp)][:, :, (m % 2) * P:(m % 2 + 1) * P],
                        rhs=w_tiles[(kd, h)][:, :,
                                             (nb - h * 4) * NW:
                                             (nb - h * 4 + 1) * NW],
                        start=(si == 0), stop=(si == len(steps) - 1),
                        perf_mode=DR)
            for j, (m, nb) in enumerate(chains):
                ot = out_pool.tile([P, NW], f32, tag="ot",
                                   name=f"ot_{m}_{nb}")
                nc.vector.tensor_copy(out=ot[:, :], in_=psums[(m, nb)][:, :])
                if len(chains) < 8:
                    # tail chunks: sync queue is idle by then
                    eng = (nc.sync, nc.scalar, nc.gpsimd)[j % 3]
                else:
                    eng = nc.scalar if j % 2 else nc.gpsimd
                eng.dma_start(
                    out=out[m * P:(m + 1) * P, nb * NW:(nb + 1) * NW],
                    in_=ot[:, :])

        # Group (mp0,h0) runs while its stream lands.  Its first output
        # DMA is the first gpsimd-queue instruction and depends on the
        # eviction tile, so the gpsimd-queue q/r DMAs for m-pairs 1..3
        # (issued right after) only start once the JIT load phase is
        # over and don't steal HBM bandwidth from it.
        run_group(0, 0)
        # tile_wait_until pushes these loads' scheduler priority past
        # G0's end so the (FIFO) gpsimd queue really orders them after
        # G0's first output DMA — without it the scheduler hoists them
        # to t=0 and they steal HBM bandwidth from the JIT stream.
        with tc.tile_wait_until(0.030):
            for mp in range(1, 4):
                for kd in range(KD):
                    load_q(kd, mp, nc.gpsimd)
                    if kd < H_CORR:
                        load_r(kd, mp, nc.gpsimd)
        for mp, h in [(0, 1), (1, 0), (1, 1), (2, 0), (2, 1), (3, 0)]:
            run_group(mp, h)
        # Final group in shrinking chunks so its eviction + output
        # DMA overlap the remaining matmuls and the tail is short.
        run_group(3, 1, nbs=[4, 5])
        run_group(3, 1, nbs=[6])
        run_group(3, 1, nbs=[7])


def _bf16_body(nc, tc, kxm, kxn, out, mm_dt, mybir):
    """Previous-best bf16 path (x^T resident, sign(w)^T streamed)."""
    P = 128
    KT = D_IN // P
    MT = M_CORE // P
    NW = 512
    NB = D_OUT // NW
    f32 = mybir.dt.float32

    from contextlib import ExitStack
    with ExitStack() as ctx:
        kxm_pool = ctx.enter_context(tc.tile_pool(name="kxm", bufs=1))
        kxn_pool = ctx.enter_context(tc.tile_pool(name="kxn", bufs=9))
        psum_pool = ctx.enter_context(
            tc.tile_pool(name="psum", bufs=8, space="PSUM"))
        out_pool = ctx.enter_context(tc.tile_pool(name="outp", bufs=8))

        def issue_chunk(nb, c, k0, sz):
            t = kxn_pool.tile([P, sz, NW], mm_dt, tag="kxn",
                              name=f"kxn_{nb}_{c}", bufs=24)
            src = kxn[k0 * P:(k0 + sz) * P, nb * NW:(nb + 1) * NW]
            nc.sync.dma_start(
                out=t, in_=src.rearrange("(ko ki) n -> ki ko n", ki=P))
            return [t[:, i, :] for i in range(sz)]

        def issue_chunks(nb, sizes):
            rhs, k0 = [], 0
            for c, sz in enumerate(sizes):
                rhs += issue_chunk(nb, c, k0, sz)
                k0 += sz
            return rhs

        kxm_tiles = {}

        def issue_kxm(k, h):
            kt = kxm_pool.tile([P, M_CORE // 2], mm_dt, tag="kxm",
                               name=f"kxm_{k}_{h}", bufs=2 * KT)
            eng = nc.scalar if h == 0 else nc.gpsimd
            eng.dma_start(out=kt[:, :],
                          in_=kxm[k * P:(k + 1) * P,
                                  h * (M_CORE // 2):(h + 1) * (M_CORE // 2)])
            kxm_tiles[(k, h)] = kt

        def lhsT(k, m):
            h, off = divmod(m, MT // 2)
            return kxm_tiles[(k, h)][:, off * P:(off + 1) * P]

        sizes0 = [2, 2, 2, 2, 4, 4, 4, 4, 4, 4]
        rhs0, k0 = [], 0
        issue_kxm(0, 0)
        issue_kxm(1, 0)
        for c, sz in enumerate(sizes0):
            rhs0 += issue_chunk(0, c, k0, sz)
            k0 += sz
            for k in range(min(k0 + 2, KT)):
                if (k, 0) not in kxm_tiles:
                    issue_kxm(k, 0)
            for k in range(min(k0 - 8, KT)):
                if (k, 1) not in kxm_tiles:
                    issue_kxm(k, 1)
        for k in range(KT):
            if (k, 0) not in kxm_tiles:
                issue_kxm(k, 0)
        for k in range(KT):
            if (k, 1) not in kxm_tiles:
                issue_kxm(k, 1)

        next_rhs = rhs0
        for nb in range(NB):
            ncols = slice(nb * NW, (nb + 1) * NW)
            rhs_k = next_rhs
            psums = [psum_pool.tile([P, NW], f32, tag="ps", name=f"ps_{nb}_{i}")
                     for i in range(MT)]
            groups = [range(MT // 2), range(MT // 2, MT)] if nb == 0 \
                else [[m] for m in range(MT)]
            for gi, ms in enumerate(groups):
                for k in range(KT):
                    for m in ms:
                        nc.tensor.matmul(
                            psums[m][:, :],
                            lhsT=lhsT(k, m),
                            rhs=rhs_k[k],
                            start=(k == 0), stop=(k == KT - 1))
                if gi == 0 and nb + 1 < NB:
                    next_rhs = issue_chunks(nb + 1, [4] * 8)
                for m in ms:
                    ot = out_pool.tile([P, NW], f32, tag="ot", name=f"ot_{nb}_{m}")
                    nc.vector.tensor_copy(out=ot[:, :], in_=psums[m][:, :])
                    nc.gpsimd.dma_start(
                        out=out[m * P:(m + 1) * P, ncols], in_=ot[:, :])


def _build():
    """Build + compile the 8-core SPMD Bass program once per process."""
    if "nc" in _cache:
        return _cache["nc"]

    import concourse.bacc as bacc
    import concourse.tile as tile
    import concourse.mybir as mybir
    from concourse.kernels.tile_matmul import matmul_tile_kernel

    mm_dt = {"f32r": mybir.dt.float32r, "bf16": mybir.dt.bfloat16,
             "fp8e4": mybir.dt.float8e4}[DTYPE]

    nc = bacc.Bacc("TRN2", target_bir_lowering=False, debug=False,
                   enable_asserts=bool(os.environ.get("BK_ASSERTS")),
                   num_devices=NCORES)
    kxr = None
    if DTYPE == "fp8e4" and IMPL == "custom":
        # Pre-tiled layouts (see _prep_inputs): row index is
        # (tile_index * 128 + partition), columns are the tile's
        # contiguous (ko, free) payload.
        kxm = nc.dram_tensor("kxm", [4 * 16 * 128, 512], mm_dt,
                             kind="ExternalInput").ap()
        kxn = nc.dram_tensor("kxn", [2 * 16 * 128, 4096], mm_dt,
                             kind="ExternalInput").ap()
        kxr = nc.dram_tensor("kxr", [4 * max(H_CORR, 1) * 128, 512], mm_dt,
                             kind="ExternalInput").ap()
    else:
        kxm = nc.dram_tensor("kxm", [D_IN, M_CORE], mm_dt,
                             kind="ExternalInput").ap()
        kxn = nc.dram_tensor("kxn", [D_IN, D_OUT], mm_dt,
                             kind="ExternalInput").ap()
    out = nc.dram_tensor("out", [M_CORE, D_OUT], mybir.dt.float32,
                         kind="ExternalOutput").ap()

    def _warmup(tc):
        # The PE clock is HAM-throttled to 1.2GHz until ~3.4us of
        # sustained matmul activity. Burn the initial DMA window
        # warming the clock gate; the PSUM bank frees on pool exit.
        from contextlib import ExitStack
        with ExitStack() as ctx:
            wp = ctx.enter_context(tc.tile_pool(name="warm", bufs=1))
            wpp = ctx.enter_context(
                tc.tile_pool(name="warmp", bufs=1, space="PSUM"))
            wdt = mybir.dt.bfloat16
            a = wp.tile([128, 128], wdt)
            b = wp.tile([128, 512], wdt)
            nc.any.memset(a[:, :], 0.0)
            nc.any.memset(b[:, :], 0.0)
            ps = wpp.tile([128, 512], mybir.dt.float32)
            for _ in range(int(os.environ.get("BK_WARM", "10"))):
                nc.tensor.matmul(ps[:, :], lhsT=a[:, :], rhs=b[:, :],
                                 start=True, stop=True)

    if IMPL == "custom" and DTYPE == "fp8e4":
        with tile.TileContext(nc) as tc:
            _warmup(tc)
            _dr_body(nc, tc, kxm, kxr, kxn, out, mybir)
    elif IMPL == "custom":
        with tile.TileContext(nc) as tc:
            _warmup(tc)
            _bf16_body(nc, tc, kxm, kxn, out, mm_dt, mybir)
    else:
        kw = {}
        if os.environ.get("BK_MAX_K_TILE"):
            kw["MAX_K_TILE_SIZE"] = int(os.environ["BK_MAX_K_TILE"])
        if os.environ.get("BK_SKIP_K_SNAKE"):
            kw["skip_k_snake"] = True
        if os.environ.get("BK_NO_CACHE_TILES"):
            kw["cache_tiles"] = False
        with tile.TileContext(nc) as tc:
            _warmup(tc)
            matmul_tile_kernel(tc, kxm, kxn, out, **kw)
    nc.compile()
    _cache["nc"] = nc
    return nc


def _prep_inputs(x, weight):
    import ml_dtypes
    if DTYPE == "bf16":
        np_dt = ml_dtypes.bfloat16
    elif DTYPE == "fp8e4":
        np_dt = ml_dtypes.float8_e4m3
    else:
        np_dt = np.float32
    x2d = np.asarray(x, dtype=np.float32).reshape(M_TOTAL, D_IN)
    sgn = np.sign(weight, dtype=np.float32).T.astype(np_dt)

    def tile5(a, nt, free):
        # [nt*256, M] -> [mp, kd, p, ko, mpw] -> [(mp kd p), 512]
        kd_n = a.shape[0] // 256
        t = a.reshape(kd_n, 2, 128, 4, free).transpose(3, 0, 2, 1, 4)
        return np.ascontiguousarray(t).reshape(4 * kd_n * 128, 2 * free)

    if DTYPE == "fp8e4" and IMPL == "custom":
        # W: [h, kd, p, ko, 2048] -> [(h kd p), 4096]
        wt = sgn.reshape(16, 2, 128, 2, 2048).transpose(3, 0, 2, 1, 4)
        kxn = np.ascontiguousarray(wt).reshape(2 * 16 * 128, 4096)
    else:
        kxn = np.ascontiguousarray(sgn)
    in_maps = []
    for c in range(NCORES):
        xs = x2d[c * M_CORE:(c + 1) * M_CORE].T  # [D_IN, M_CORE]
        q8 = xs.astype(np_dt)
        if DTYPE == "fp8e4" and IMPL == "custom":
            kc = max(H_CORR, 1) * 256
            resid = (xs[:kc] - q8[:kc].astype(np.float32)).astype(np_dt)
            if H_CORR == 0:
                resid[:] = 0
            im = {"kxm": tile5(q8, 16, 256),
                  "kxr": tile5(resid, max(H_CORR, 1), 256),
                  "kxn": kxn}
        else:
            im = {"kxm": np.ascontiguousarray(q8), "kxn": kxn}
        in_maps.append(im)
    return in_maps


def _run(x, weight, bias, trace=False):
    from concourse.bass_utils import run_bass_kernel_spmd

    nc = _build()
    in_maps = _prep_inputs(x, weight)
    res = run_bass_kernel_spmd(nc, in_maps, core_ids=list(range(NCORES)),
                               trace=trace)
    out = np.concatenate([res.results[c]["out"] for c in range(NCORES)],
                         axis=0)
    bias = np.asarray(bias, dtype=np.float32)
    if np.any(bias):
        out += bias
    return out.reshape(B, S, D_OUT), res


def kernel(x, weight, bias):
    out, _ = _run(x, weight, bias, trace=False)
    return out
